# revision 1
# baseline (speedup 1.0000x reference)
"""Trainium2 Bass kernel for GCFAgg-style block:
    q1 = x@W1.T+b1; q2 = x@W2.T+b2; r = x@WR.T+br
    out = (q1 @ q2.T) @ r        (per batch, no softmax)

Key algebraic restructuring: with x_aug = [x | 1] and W*_aug = [W* | b*],
    out = x_aug @ (Khat @ (x_aug.T @ x_aug) @ Rhat)
where Khat = W1_aug.T @ W2_aug and Rhat = WR_aug.T are tiny host-precomputed
matrices. The device only computes G = x.T @ x (per batch, symmetric — only
upper blocks are computed, lower blocks come from PE transposes) plus a small
[640]^2-sized chain and the final projection out = x @ P + v. That's
~3.5 GFLOP/core instead of ~41 GFLOP/core for the naive N x N similarity
materialization. The augmented row/col of G (column sums of x) and the
constant v-broadcast are folded in from host-precomputed side inputs.

Numerics: fp32r matmuls (fp32 storage, single-pass reduced-precision PE
multiply) giving ~2e-4 relative error vs the fp32 reference — ~10x tighter
than bf16 at only ~10-15% more device time.

Sharding: batch dim B=8, one batch per NeuronCore (data parallel, 8 cores).

Self-contained: hardcodes shapes from the problem spec
(x: [8, 4096, 512] f32; W*: [512, 512]; b*: [512]).
"""
import os
import sys

sys.path.insert(0, "/opt/trn_rl_repo")

import numpy as np
import ml_dtypes

import concourse.bass as bass
import concourse.mybir as mybir
import concourse.tile as tile
from concourse import bacc
from concourse.bass_utils import run_bass_kernel_spmd
from concourse.masks import make_identity
from concourse.tile_rust import add_dep_helper

B = 8          # batch -> one per core
N = 4096       # tokens per batch
D = 512        # model dim
GP = 640       # augmented dim 513 padded to 5*128
NCHUNK = GP // 128   # 5
NT = N // 128        # 32 row tiles
N_CORES = 8

F32 = mybir.dt.float32
F32R = mybir.dt.float32r
BF16 = mybir.dt.bfloat16

# mode: "f32r" (fp32 storage, single-pass reduced-precision matmul),
#       "bf16" (bf16 storage+matmul), "f32" (full-precision 4-pass matmul)
MODE = os.environ.get("GCF_MODE", "f32r")

_built = {}


def _build(mode):
    if mode in _built:
        return _built[mode]

    # Storage dtype IS the matmul dtype: the BIR verifier requires fp32r
    # matmul inputs to be produced (DMA'd/copied) as fp32r.
    if mode == "bf16":
        big_mm = BF16
    elif mode == "f32":
        big_mm = F32
    else:
        big_mm = F32R
    big_store = big_mm
    chain_mm = F32 if mode == "f32" else F32R

    def mm_ap(ap, dt):
        return ap if ap.dtype == dt else ap.bitcast(dt)

    nc = bacc.Bacc("TRN2", target_bir_lowering=False, debug=False,
                   num_devices=N_CORES)

    xa_d = nc.dram_tensor("xa", (N, D), big_store, kind="ExternalInput")
    xat_d = nc.dram_tensor("xat", (NT, 128, 4, 128), big_store,
                           kind="ExternalInput")
    khatT_d = nc.dram_tensor("khatT", (GP, GP), chain_mm, kind="ExternalInput")
    rhat_d = nc.dram_tensor("rhat", (GP, D), chain_mm, kind="ExternalInput")
    # host-precomputed augmented pieces of G_aug (they only involve column
    # sums of x, cheap on host): rows 512:640, and the [:, 512:640] blocks
    gext_d = nc.dram_tensor("gext", (128, GP), chain_mm, kind="ExternalInput")
    augblk_d = nc.dram_tensor("augblk", (4, 128, GP - D), chain_mm,
                              kind="ExternalInput")
    m1row_d = nc.dram_tensor("m1row", (1, D), chain_mm, kind="ExternalInput")
    out_d = nc.dram_tensor("out", (N, D), F32, kind="ExternalOutput")

    with tile.TileContext(nc) as tc:
        with (
            tc.tile_pool(name="xa", bufs=16) as xa_pool,
            tc.tile_pool(name="xat", bufs=32) as xat_pool,
            tc.tile_pool(name="const", bufs=1) as const_pool,
            tc.tile_pool(name="gsb", bufs=1) as g_pool,
            tc.tile_pool(name="chain", bufs=1) as chain_pool,
            tc.tile_pool(name="outsb", bufs=6) as out_pool,
        ):
            # ---- constants (via the otherwise-idle GpSimd DMA queue so they
            # don't delay the sync-queue xa/xat streams) ----
            khat_sb = [const_pool.tile([128, GP], chain_mm, tag=f"khat{c}",
                                       name=f"khat{c}") for c in range(NCHUNK)]
            rhat_sb = [const_pool.tile([128, D], chain_mm, tag=f"rhat{c}",
                                       name=f"rhat{c}") for c in range(NCHUNK)]
            ident = const_pool.tile([128, 128], F32, tag="ident")
            make_identity(nc, ident[:])
            # dtype for the K=1 v-broadcast matmul: bitcasting f32r->f32 is
            # size-preserving, but bf16 tiles must stay bf16 (memset can emit
            # bf16/f32, just not f32r)
            v_mm_dt = big_mm if big_mm == BF16 else F32
            ones_row = const_pool.tile([1, 128], v_mm_dt, tag="ones_row")
            nc.vector.memset(ones_row[:], 1.0)

            # ---- phase 1: G = x^T @ x over 32 row tiles; G is symmetric so
            # only the upper block-triangle is computed on PE ----
            g_sb = [g_pool.tile([128, GP], chain_mm, tag=f"g{c}", name=f"g{c}")
                    for c in range(NCHUNK)]
            with tc.tile_pool(name="psG", bufs=1, space="PSUM") as psG_pool:
                ps_ga = [psG_pool.tile([128, D - c * 128], F32, tag=f"ga{c}",
                                       name=f"ga{c}") for c in range(4)]
                gate_mms = []
                for t in range(NT):
                    xa_t = xa_pool.tile([128, D], big_store, tag="xa")
                    nc.sync.dma_start(xa_t[:], xa_d.ap()[t * 128:(t + 1) * 128, :])
                    for c in range(4):
                        mm = nc.tensor.matmul(
                            ps_ga[c][:],
                            mm_ap(xa_t[:, c * 128:(c + 1) * 128], big_mm),
                            mm_ap(xa_t[:, c * 128:D], big_mm),
                            start=(t == 0), stop=(t == NT - 1),
                        )
                        if c == 3:
                            gate_mms.append(mm)
                gate_mm16 = gate_mms[16]
                # constants + host-side G_aug pieces: gated behind mid-G so
                # their DMAs don't compete with the xa stream during warmup
                # (they're first needed at chain time)
                const_dmas = []
                const_dmas.append(nc.gpsimd.dma_start(g_sb[4][:], gext_d.ap()[:]))
                for c in range(4):
                    const_dmas.append(
                        nc.gpsimd.dma_start(g_sb[c][:, D:GP], augblk_d.ap()[c]))
                m1row_sb = const_pool.tile([1, D], chain_mm, tag="m1row")
                const_dmas.append(nc.gpsimd.dma_start(m1row_sb[:], m1row_d.ap()[:]))
                for c in range(NCHUNK):
                    const_dmas.append(nc.gpsimd.dma_start(
                        khat_sb[c][:], khatT_d.ap()[c * 128:(c + 1) * 128, :]))
                    const_dmas.append(nc.gpsimd.dma_start(
                        rhat_sb[c][:], rhat_d.ap()[c * 128:(c + 1) * 128, :]))
                for cd in const_dmas:
                    add_dep_helper(cd.ins, gate_mm16.ins,
                                   reason="const loads gated behind G t=16")
                # upper blocks into SBUF
                for c in range(4):
                    nc.vector.tensor_copy(g_sb[c][:, c * 128:D], ps_ga[c][:])
                # lower blocks = transpose of upper (G symmetric)
                for c2 in range(1, 4):
                    for c1 in range(c2):
                        ps_tr = psG_pool.tile([128, 128], F32, tag="tr", bufs=2)
                        nc.tensor.transpose(
                            ps_tr[:],
                            mm_ap(g_sb[c1][:, c2 * 128:(c2 + 1) * 128], F32),
                            ident[:],
                        )
                        nc.vector.tensor_copy(
                            g_sb[c2][:, c1 * 128:(c1 + 1) * 128], ps_tr[:])

            # ---- phase 2: P = Khat @ G @ Rhat  (small chain) ----
            with tc.tile_pool(name="psC", bufs=2, space="PSUM") as psC_pool:
                # M1 rows 512:640 come from host (m1row = sx_aug @ Rhat);
                # device computes chunks 0..3 only
                m1_sb = [chain_pool.tile([128, D], chain_mm, tag=f"m1{c}",
                                         name=f"m1{c}") for c in range(4)]
                for g1 in range(4):
                    ps = psC_pool.tile([128, D], F32, tag="chain", bufs=3)
                    for g2 in range(NCHUNK):
                        nc.tensor.matmul(
                            ps[:],
                            mm_ap(g_sb[g2][:, g1 * 128:(g1 + 1) * 128], chain_mm),
                            mm_ap(rhat_sb[g2][:], chain_mm),
                            start=(g2 == 0), stop=(g2 == NCHUNK - 1),
                        )
                    nc.vector.tensor_copy(m1_sb[g1][:], ps[:])

                p_sb = [chain_pool.tile([128, D], big_store, tag=f"p{c}",
                                        name=f"p{c}") for c in range(NCHUNK)]
                for g1 in range(NCHUNK):
                    ps = psC_pool.tile([128, D], F32, tag="chain", bufs=3)
                    for g2 in range(4):
                        nc.tensor.matmul(
                            ps[:],
                            mm_ap(khat_sb[g2][:, g1 * 128:(g1 + 1) * 128], chain_mm),
                            mm_ap(m1_sb[g2][:], chain_mm),
                            start=(g2 == 0), stop=False,
                        )
                    # g2=4 contribution: only row 512 of K^T/M1 is nonzero
                    nc.tensor.matmul(
                        ps[:],
                        mm_ap(khat_sb[4][0:1, g1 * 128:(g1 + 1) * 128], chain_mm),
                        mm_ap(m1row_sb[0:1, :], chain_mm),
                        start=False, stop=True,
                    )
                    nc.vector.tensor_copy(p_sb[g1][:], ps[:])

            # ---- phase 3: out = x @ P[0:512] + v,  v = P_aug[512, :] ----
            with tc.tile_pool(name="psO", bufs=1, space="PSUM") as psO_pool:
                # v broadcast to 128 partitions via a K=1 fp32 matmul
                ps_v = psO_pool.tile([128, D], F32, tag="v", bufs=1)
                nc.tensor.matmul(
                    ps_v[:], ones_row[0:1, :], mm_ap(p_sb[4][0:1, :], v_mm_dt),
                    start=True, stop=True,
                )
                v_sb = const_pool.tile([128, D], F32, tag="vsb")
                nc.vector.tensor_copy(v_sb[:], ps_v[:])

                for t in range(NT):
                    xat_t = xat_pool.tile([128, 4, 128], big_store, tag="xat")
                    xdma = nc.scalar.dma_start(xat_t[:], xat_d.ap()[t])
                    # full xat residency, bandwidth-shaped: the xa stream alone
                    # needs ~190GB/s of the ~340GB/s during G, so release xat
                    # at only 1 tile per 2 G tiles there; the remainder streams
                    # during the chain window, which otherwise runs at ~60% BW
                    add_dep_helper(xdma.ins, gate_mms[min(NT - 1, 2 * t + 6)].ins,
                                   reason="xat prefetch BW-shaped behind G")
                    ps = psO_pool.tile([128, D], F32, tag="out", bufs=6)
                    for c in range(4):
                        nc.tensor.matmul(
                            ps[:],
                            mm_ap(xat_t[:, c, :], big_mm),
                            mm_ap(p_sb[c][:], big_mm),
                            start=(c == 0), stop=(c == 3),
                        )
                    ot = out_pool.tile([128, D], F32, tag="ot")
                    nc.vector.tensor_add(ot[:], ps[:], v_sb[:])
                    # alternate store triggers across two queues: a single
                    # queue serializes 32 x ~640ns DMA_DIRECT2D triggers
                    eng = nc.gpsimd if t % 2 == 0 else nc.sync
                    eng.dma_start(out_d.ap()[t * 128:(t + 1) * 128, :], ot[:])

    nc.compile()
    _built[mode] = nc
    return nc


def _prep_host(x, Wq1_w, Wq1_b, Wq2_w, Wq2_b, WR_w, WR_b, mode):
    f = np.float32
    W1a = np.concatenate([Wq1_w, Wq1_b[:, None]], axis=1)   # [512, 513]
    W2a = np.concatenate([Wq2_w, Wq2_b[:, None]], axis=1)
    WRa = np.concatenate([WR_w, WR_b[:, None]], axis=1)

    khatT = np.zeros((GP, GP), f)   # Khat^T = W2a^T @ W1a, padded
    khatT[:D + 1, :D + 1] = (
        W2a.T.astype(np.float64) @ W1a.astype(np.float64)
    ).astype(f)
    rhat = np.zeros((GP, D), f)     # Rhat = WRa^T, padded
    rhat[:D + 1, :] = WRa.T

    # augmented pieces of G_aug = xa^T @ xa that only need column sums of x
    sx = x.sum(axis=1, dtype=np.float64).astype(f)       # [B, 512]
    gext = np.zeros((B, 128, GP), f)                     # G_aug rows 512:640
    gext[:, 0, :D] = sx
    gext[:, 0, D] = float(N)
    augblk = np.zeros((B, 4, 128, GP - D), f)            # G_aug[:512, 512:640]
    augblk[:, :, :, 0] = sx.reshape(B, 4, 128)
    # M1 row 512 = sx_aug @ Rhat (fully host-computable)
    sxa = np.concatenate([sx, np.full((B, 1), float(N), f)], axis=1)  # [B, 513]
    m1row = (sxa.astype(np.float64) @ WRa.T.astype(np.float64)).astype(f)[:, None, :]

    # xat[b, t, p, c, j] = x[b, t*128+j, c*128+p] — per-(t) contiguous
    # [128, 4, 128] lhsT blocks of x^T
    xat = np.ascontiguousarray(
        x.transpose(0, 2, 1)                     # [B, 512, 4096]
         .reshape(B, 4, 128, NT, 128)            # [B, c, p, t, j]
         .transpose(0, 3, 2, 1, 4)               # [B, t, p, c, j]
    )
    xa = x

    if mode == "bf16":
        bf = ml_dtypes.bfloat16
        xa = xa.astype(bf)
        xat = xat.astype(bf)
    else:
        xa = np.ascontiguousarray(xa)
    return xa, xat, khatT, rhat, gext, augblk, m1row


def kernel(x, Wq1_w, Wq1_b, Wq2_w, Wq2_b, WR_w, WR_b):
    x = np.asarray(x, dtype=np.float32)
    args = [np.asarray(a, dtype=np.float32)
            for a in (Wq1_w, Wq1_b, Wq2_w, Wq2_b, WR_w, WR_b)]
    xa, xat, khatT, rhat, gext, augblk, m1row = _prep_host(x, *args, MODE)

    nc = _build(MODE)
    in_maps = [
        {"xa": xa[b], "xat": xat[b], "khatT": khatT, "rhat": rhat,
         "gext": gext[b], "augblk": augblk[b], "m1row": m1row[b]}
        for b in range(B)
    ]
    # the axon-tunneled device occasionally starts in a wedged state
    # (NRT_EXEC_UNIT_UNRECOVERABLE) and recovers on the next attempt
    last_err = None
    for attempt in range(3):
        try:
            res = run_bass_kernel_spmd(nc, in_maps, core_ids=list(range(N_CORES)))
            break
        except Exception as e:  # noqa: BLE001
            last_err = e
            import time as _time
            _time.sleep(2.0)
            try:
                import jax
                jax.clear_caches()
            except Exception:
                pass
    else:
        raise last_err
    return np.stack([res.results[b]["out"] for b in range(B)])



# revision 2
# speedup vs baseline: 1.0818x; 1.0818x over previous
"""Trainium2 Bass kernel for GCFAgg-style block:
    q1 = x@W1.T+b1; q2 = x@W2.T+b2; r = x@WR.T+br
    out = (q1 @ q2.T) @ r        (per batch, no softmax)

Key algebraic restructuring: with x_aug = [x | 1] and W*_aug = [W* | b*],
    out = x_aug @ (Khat @ (x_aug.T @ x_aug) @ Rhat)
where Khat = W1_aug.T @ W2_aug and Rhat = WR_aug.T are tiny host-precomputed
matrices. The device only computes G = x.T @ x (per batch, symmetric — only
upper blocks are computed, lower blocks come from PE transposes) plus a small
[640]^2-sized chain and the final projection out = x @ P + v. That's
~3.5 GFLOP/core instead of ~41 GFLOP/core for the naive N x N similarity
materialization. The augmented row of G (column sums of x) is a 1-row
host-precomputed side input; the augmented column of G never feeds any
matmul slice (M1 columns stop at 511) so it is not loaded at all.

Perf model (per core, PE @2.4GHz, DMA ~332GB/s eff):
  PE:  G 41k cyc + chain ~21k + out 65.5k  ~= 128k cyc = 53us  <- binding
  DMA (bf16 x streams + f32 out + compact consts): ~18MB = 55us, overlapped
bf16/f32r matmul both run 1 cycle/row (free>=256), so bf16 only buys DMA
bytes; the chain stays f32r for accuracy.

Sharding: batch dim B=8, one batch per NeuronCore (data parallel, 8 cores).

Self-contained: hardcodes shapes from the problem spec
(x: [8, 4096, 512] f32; W*: [512, 512]; b*: [512]).
"""
import os
import sys

sys.path.insert(0, "/opt/trn_rl_repo")

import numpy as np
import ml_dtypes

import concourse.bass as bass
import concourse.mybir as mybir
import concourse.tile as tile
from concourse import bacc
from concourse.bass_utils import run_bass_kernel_spmd
from concourse.masks import make_identity
from concourse.tile_rust import add_dep_helper

B = 8          # batch -> one per core
N = 4096       # tokens per batch
D = 512        # model dim
GP = 640       # augmented dim 513 padded to 5*128
NCHUNK = GP // 128   # 5
NT = N // 128        # 32 row tiles
N_CORES = 8

F32 = mybir.dt.float32
F32R = mybir.dt.float32r
BF16 = mybir.dt.bfloat16

# mode: "bf16" (bf16 x/P storage+matmul, f32r chain),
#       "f32r" (fp32 storage, single-pass reduced-precision matmul),
#       "f32"  (full-precision 4-pass matmul)
MODE = os.environ.get("GCF_MODE", "bf16")

_built = {}


def _build(mode):
    if mode in _built:
        return _built[mode]

    # Storage dtype IS the matmul dtype: the BIR verifier requires fp32r
    # matmul inputs to be produced (DMA'd/copied) as fp32r.
    if mode == "bf16":
        big_mm = BF16
    elif mode == "f32":
        big_mm = F32
    else:
        big_mm = F32R
    big_store = big_mm
    chain_mm = F32 if mode == "f32" else F32R

    def mm_ap(ap, dt):
        return ap if ap.dtype == dt else ap.bitcast(dt)

    nc = bacc.Bacc("TRN2", target_bir_lowering=False, debug=False,
                   num_devices=N_CORES)

    xa_d = nc.dram_tensor("xa", (N, D), big_store, kind="ExternalInput")
    xat_d = nc.dram_tensor("xat", (NT, 128, 4, 128), big_store,
                           kind="ExternalInput")
    khatT_d = nc.dram_tensor("khatT", (D, GP), chain_mm, kind="ExternalInput")
    khat4_d = nc.dram_tensor("khat4", (1, GP), chain_mm, kind="ExternalInput")
    rhat_d = nc.dram_tensor("rhat", (D, D), chain_mm, kind="ExternalInput")
    rhat4_d = nc.dram_tensor("rhat4", (1, D), chain_mm, kind="ExternalInput")
    # host-precomputed augmented row of G_aug (row 512 = [colsums(x), N]):
    # the only augmented piece the chain actually reads
    gext_d = nc.dram_tensor("gext", (1, GP), chain_mm, kind="ExternalInput")
    m1row_d = nc.dram_tensor("m1row", (1, D), chain_mm, kind="ExternalInput")
    out_d = nc.dram_tensor("out", (N, D), F32, kind="ExternalOutput")

    with tile.TileContext(nc) as tc:
        with (
            tc.tile_pool(name="xa", bufs=16) as xa_pool,
            tc.tile_pool(name="xat", bufs=32) as xat_pool,
            tc.tile_pool(name="const", bufs=1) as const_pool,
            tc.tile_pool(name="gsb", bufs=1) as g_pool,
            tc.tile_pool(name="chain", bufs=1) as chain_pool,
            tc.tile_pool(name="outsb", bufs=6) as out_pool,
        ):
            # ---- constants (via the otherwise-idle GpSimd DMA queue so they
            # don't delay the sync-queue xa/xat streams) ----
            khat_sb = [const_pool.tile([128, GP], chain_mm, tag=f"khat{c}",
                                       name=f"khat{c}") for c in range(4)]
            khat4_sb = const_pool.tile([1, GP], chain_mm, tag="khat4")
            rhat_sb = [const_pool.tile([128, D], chain_mm, tag=f"rhat{c}",
                                       name=f"rhat{c}") for c in range(4)]
            rhat4_sb = const_pool.tile([1, D], chain_mm, tag="rhat4")
            gext_sb = const_pool.tile([1, GP], chain_mm, tag="gext")
            ident = const_pool.tile([128, 128], F32, tag="ident")
            make_identity(nc, ident[:])
            # dtype for the K=1 v-broadcast matmul: bitcasting f32r->f32 is
            # size-preserving, but bf16 tiles must stay bf16 (memset can emit
            # bf16/f32, just not f32r)
            v_mm_dt = big_mm if big_mm == BF16 else F32
            ones_row = const_pool.tile([1, 128], v_mm_dt, tag="ones_row")
            nc.vector.memset(ones_row[:], 1.0)

            # ---- phase 1: G = x^T @ x over 32 row tiles; G is symmetric so
            # only the upper block-triangle is computed on PE ----
            g_sb = [g_pool.tile([128, D], chain_mm, tag=f"g{c}", name=f"g{c}")
                    for c in range(4)]
            with tc.tile_pool(name="psG", bufs=1, space="PSUM") as psG_pool:
                ps_ga = [psG_pool.tile([128, D - c * 128], F32, tag=f"ga{c}",
                                       name=f"ga{c}") for c in range(4)]
                gate_mms = []
                for t in range(NT):
                    xa_t = xa_pool.tile([128, D], big_store, tag="xa")
                    nc.sync.dma_start(xa_t[:], xa_d.ap()[t * 128:(t + 1) * 128, :])
                    for c in range(4):
                        mm = nc.tensor.matmul(
                            ps_ga[c][:],
                            mm_ap(xa_t[:, c * 128:(c + 1) * 128], big_mm),
                            mm_ap(xa_t[:, c * 128:D], big_mm),
                            start=(t == 0), stop=(t == NT - 1),
                        )
                        if c == 3:
                            gate_mms.append(mm)
                gate_mm16 = gate_mms[16]
                # constants: gated behind mid-G so their DMAs don't compete
                # with the xa stream during warmup (first needed at chain time)
                const_dmas = []
                const_dmas.append(nc.gpsimd.dma_start(gext_sb[:], gext_d.ap()[:]))
                const_dmas.append(nc.gpsimd.dma_start(khat4_sb[:], khat4_d.ap()[:]))
                const_dmas.append(nc.gpsimd.dma_start(rhat4_sb[:], rhat4_d.ap()[:]))
                m1row_sb = const_pool.tile([1, D], chain_mm, tag="m1row")
                const_dmas.append(nc.gpsimd.dma_start(m1row_sb[:], m1row_d.ap()[:]))
                for c in range(4):
                    const_dmas.append(nc.gpsimd.dma_start(
                        khat_sb[c][:], khatT_d.ap()[c * 128:(c + 1) * 128, :]))
                    const_dmas.append(nc.gpsimd.dma_start(
                        rhat_sb[c][:], rhat_d.ap()[c * 128:(c + 1) * 128, :]))
                for cd in const_dmas:
                    add_dep_helper(cd.ins, gate_mm16.ins,
                                   reason="const loads gated behind G t=16")
                # upper blocks into SBUF
                for c in range(4):
                    nc.vector.tensor_copy(g_sb[c][:, c * 128:D], ps_ga[c][:])
                # lower blocks = transpose of upper (G symmetric)
                for c2 in range(1, 4):
                    for c1 in range(c2):
                        ps_tr = psG_pool.tile([128, 128], F32, tag="tr", bufs=2)
                        nc.tensor.transpose(
                            ps_tr[:],
                            mm_ap(g_sb[c1][:, c2 * 128:(c2 + 1) * 128], F32),
                            ident[:],
                        )
                        nc.vector.tensor_copy(
                            g_sb[c2][:, c1 * 128:(c1 + 1) * 128], ps_tr[:])

            # ---- phase 2: P = Khat @ G @ Rhat  (small chain) ----
            with tc.tile_pool(name="psC", bufs=2, space="PSUM") as psC_pool:
                # M1 rows 512:640 come from host (m1row = sx_aug @ Rhat);
                # device computes chunks 0..3 only. The G-row-512 term enters
                # as a K=1 matmul (gext row x rhat row 512).
                m1_sb = [chain_pool.tile([128, D], chain_mm, tag=f"m1{c}",
                                         name=f"m1{c}") for c in range(4)]
                for g1 in range(4):
                    ps = psC_pool.tile([128, D], F32, tag="chain", bufs=3)
                    for g2 in range(4):
                        nc.tensor.matmul(
                            ps[:],
                            mm_ap(g_sb[g2][:, g1 * 128:(g1 + 1) * 128], chain_mm),
                            mm_ap(rhat_sb[g2][:], chain_mm),
                            start=(g2 == 0), stop=False,
                        )
                    nc.tensor.matmul(
                        ps[:],
                        mm_ap(gext_sb[0:1, g1 * 128:(g1 + 1) * 128], chain_mm),
                        mm_ap(rhat4_sb[0:1, :], chain_mm),
                        start=False, stop=True,
                    )
                    nc.vector.tensor_copy(m1_sb[g1][:], ps[:])

                p_sb = [chain_pool.tile([128, D], big_store, tag=f"p{c}",
                                        name=f"p{c}") for c in range(NCHUNK)]
                for g1 in range(NCHUNK):
                    ps = psC_pool.tile([128, D], F32, tag="chain", bufs=3)
                    for g2 in range(4):
                        nc.tensor.matmul(
                            ps[:],
                            mm_ap(khat_sb[g2][:, g1 * 128:(g1 + 1) * 128], chain_mm),
                            mm_ap(m1_sb[g2][:], chain_mm),
                            start=(g2 == 0), stop=False,
                        )
                    # g2=4 contribution: only row 512 of K^T/M1 is nonzero
                    nc.tensor.matmul(
                        ps[:],
                        mm_ap(khat4_sb[0:1, g1 * 128:(g1 + 1) * 128], chain_mm),
                        mm_ap(m1row_sb[0:1, :], chain_mm),
                        start=False, stop=True,
                    )
                    nc.vector.tensor_copy(p_sb[g1][:], ps[:])

            # ---- phase 3: out = x @ P[0:512] + v,  v = P_aug[512, :] ----
            with tc.tile_pool(name="psO", bufs=1, space="PSUM") as psO_pool:
                # v broadcast to 128 partitions via a K=1 fp32 matmul
                ps_v = psO_pool.tile([128, D], F32, tag="v", bufs=1)
                nc.tensor.matmul(
                    ps_v[:], ones_row[0:1, :], mm_ap(p_sb[4][0:1, :], v_mm_dt),
                    start=True, stop=True,
                )
                v_sb = const_pool.tile([128, D], F32, tag="vsb")
                nc.vector.tensor_copy(v_sb[:], ps_v[:])

                for t in range(NT):
                    xat_t = xat_pool.tile([128, 4, 128], big_store, tag="xat")
                    xdma = nc.scalar.dma_start(xat_t[:], xat_d.ap()[t])
                    # full xat residency, bandwidth-shaped: release xat tiles
                    # progressively behind the G stream so they never starve
                    # the xa loads; the remainder streams during the chain
                    # window, which otherwise leaves DMA idle
                    add_dep_helper(xdma.ins, gate_mms[min(NT - 1, t + 4)].ins,
                                   reason="xat prefetch BW-shaped behind G")
                    ps = psO_pool.tile([128, D], F32, tag="out", bufs=6)
                    for c in range(4):
                        nc.tensor.matmul(
                            ps[:],
                            mm_ap(xat_t[:, c, :], big_mm),
                            mm_ap(p_sb[c][:], big_mm),
                            start=(c == 0), stop=(c == 3),
                        )
                    ot = out_pool.tile([128, D], F32, tag="ot")
                    nc.vector.tensor_add(ot[:], ps[:], v_sb[:])
                    # alternate store triggers across two queues: a single
                    # queue serializes 32 x ~640ns DMA_DIRECT2D triggers
                    eng = nc.gpsimd if t % 2 == 0 else nc.sync
                    eng.dma_start(out_d.ap()[t * 128:(t + 1) * 128, :], ot[:])

    nc.compile()
    _built[mode] = nc
    return nc


def _prep_host(x, Wq1_w, Wq1_b, Wq2_w, Wq2_b, WR_w, WR_b, mode):
    f = np.float32
    W1a = np.concatenate([Wq1_w, Wq1_b[:, None]], axis=1)   # [512, 513]
    W2a = np.concatenate([Wq2_w, Wq2_b[:, None]], axis=1)
    WRa = np.concatenate([WR_w, WR_b[:, None]], axis=1)

    khatT_full = np.zeros((GP, GP), f)   # Khat^T = W2a^T @ W1a, padded
    khatT_full[:D + 1, :D + 1] = (
        W2a.T.astype(np.float64) @ W1a.astype(np.float64)
    ).astype(f)
    khatT = np.ascontiguousarray(khatT_full[:D])            # rows 0:512
    khat4 = np.ascontiguousarray(khatT_full[D:D + 1])       # row 512
    rhat_full = np.zeros((GP, D), f)     # Rhat = WRa^T, padded
    rhat_full[:D + 1, :] = WRa.T
    rhat = np.ascontiguousarray(rhat_full[:D])
    rhat4 = np.ascontiguousarray(rhat_full[D:D + 1])

    # augmented row of G_aug = xa^T @ xa (row 512 = [colsums(x), N, 0..])
    sx = x.sum(axis=1, dtype=np.float64).astype(f)       # [B, 512]
    gext = np.zeros((B, 1, GP), f)
    gext[:, 0, :D] = sx
    gext[:, 0, D] = float(N)
    # M1 row 512 = sx_aug @ Rhat (fully host-computable)
    sxa = np.concatenate([sx, np.full((B, 1), float(N), f)], axis=1)  # [B, 513]
    m1row = (sxa.astype(np.float64) @ WRa.T.astype(np.float64)).astype(f)[:, None, :]

    # xat[b, t, p, c, j] = x[b, t*128+j, c*128+p] — per-(t) contiguous
    # [128, 4, 128] lhsT blocks of x^T
    xat = np.ascontiguousarray(
        x.transpose(0, 2, 1)                     # [B, 512, 4096]
         .reshape(B, 4, 128, NT, 128)            # [B, c, p, t, j]
         .transpose(0, 3, 2, 1, 4)               # [B, t, p, c, j]
    )
    xa = x

    if mode == "bf16":
        bf = ml_dtypes.bfloat16
        xa = xa.astype(bf)
        xat = xat.astype(bf)
    else:
        xa = np.ascontiguousarray(xa)
    return xa, xat, khatT, khat4, rhat, rhat4, gext, m1row


def kernel(x, Wq1_w, Wq1_b, Wq2_w, Wq2_b, WR_w, WR_b):
    x = np.asarray(x, dtype=np.float32)
    args = [np.asarray(a, dtype=np.float32)
            for a in (Wq1_w, Wq1_b, Wq2_w, Wq2_b, WR_w, WR_b)]
    xa, xat, khatT, khat4, rhat, rhat4, gext, m1row = _prep_host(x, *args, MODE)

    nc = _build(MODE)
    in_maps = [
        {"xa": xa[b], "xat": xat[b], "khatT": khatT, "khat4": khat4,
         "rhat": rhat, "rhat4": rhat4, "gext": gext[b], "m1row": m1row[b]}
        for b in range(B)
    ]
    # the axon-tunneled device occasionally starts in a wedged state
    # (NRT_EXEC_UNIT_UNRECOVERABLE) and recovers on the next attempt
    last_err = None
    for attempt in range(3):
        try:
            res = run_bass_kernel_spmd(nc, in_maps, core_ids=list(range(N_CORES)))
            break
        except Exception as e:  # noqa: BLE001
            last_err = e
            import time as _time
            _time.sleep(2.0)
            try:
                import jax
                jax.clear_caches()
            except Exception:
                pass
    else:
        raise last_err
    return np.stack([res.results[b]["out"] for b in range(B)])


# revision 3
# speedup vs baseline: 1.1277x; 1.0423x over previous
"""Trainium2 Bass kernel for GCFAgg-style block:
    q1 = x@W1.T+b1; q2 = x@W2.T+b2; r = x@WR.T+br
    out = (q1 @ q2.T) @ r        (per batch, no softmax)

Key algebraic restructuring: with x_aug = [x | 1] and W*_aug = [W* | b*],
    out = x_aug @ (Khat @ (x_aug.T @ x_aug) @ Rhat)
where Khat = W1_aug.T @ W2_aug and Rhat = WR_aug.T are tiny host-precomputed
matrices. The device computes G = x.T @ x (symmetric: upper block-triangle
on PE, lower from PE transposes), the small chain P = Khat @ G @ Rhat, and
the projection out.T = P[:512].T @ x.T + v (v = P_aug row 512, which only
needs O(N*D) host work: v = WRa @ [x.T z | sum z], z = x@k + k_512).

Perf notes (per core, PE @2.4GHz, ~332GB/s HBM):
  - PE floor ~127k cycles = 53us (G 41k, chain 20.5k, out 65.5k): binding.
  - bf16 x streams and P: bf16/f32r matmul are both 1 cycle/row, so bf16
    only buys DMA bytes. Chain stays f32r for accuracy.
  - DMA triggers serialize on their queue engine (~max(500ns, bytes/BW)
    each), so tiles are moved in batched triggers (4 row-tiles each) and
    spread across queues: xa on sync (+first tile on gpsimd, whose preamble
    ends ~1.2us earlier), xat on sync gated behind G (the chain window is
    otherwise DMA-idle), consts on gpsimd, stores on gpsimd.
  - out.T orientation makes +v a per-partition bias, fused into the
    PSUM->SBUF copy on the otherwise-idle Scalar engine (saves ~22us of
    Vector adds); host reassembles the transposed output.

Sharding: batch dim B=8, one batch per NeuronCore (data parallel).

Self-contained: hardcodes shapes from the problem spec
(x: [8, 4096, 512] f32; W*: [512, 512]; b*: [512]).
"""
import os
import sys

sys.path.insert(0, "/opt/trn_rl_repo")

import numpy as np
import ml_dtypes

import concourse.bass as bass
import concourse.mybir as mybir
import concourse.tile as tile
from concourse import bacc
from concourse.bass_utils import run_bass_kernel_spmd
from concourse.masks import make_identity
from concourse.tile_rust import add_dep_helper

B = 8          # batch -> one per core
N = 4096       # tokens per batch
D = 512        # model dim
NT = N // 128  # 32 row tiles
NSLAB = 8      # 512-token slabs for the out.T phase
N_CORES = 8

F32 = mybir.dt.float32
F32R = mybir.dt.float32r
BF16 = mybir.dt.bfloat16

# mode: "bf16" (bf16 x/P storage+matmul, f32r chain) or "f32r"
MODE = os.environ.get("GCF_MODE", "bf16")

# xa trigger batching: first single tile goes on gpsimd (earliest-ready
# queue), the rest stream on sync
XA_BATCHES = [1, 1, 2, 4, 4, 4, 4, 4, 4, 4]

_built = {}


def _build(mode):
    if mode in _built:
        return _built[mode]

    big = BF16 if mode == "bf16" else F32R
    chain = F32R

    def mm_ap(ap, dt):
        return ap if ap.dtype == dt else ap.bitcast(dt)

    nc = bacc.Bacc("TRN2", target_bir_lowering=False, debug=False,
                   num_devices=N_CORES)

    # all DRAM tensors are laid out partition-dim first by the host
    xa_d = nc.dram_tensor("xa", (128, NT, D), big, kind="ExternalInput")
    xat_d = nc.dram_tensor("xat", (4, 128, NSLAB, D), big, kind="ExternalInput")
    khat_d = nc.dram_tensor("khat", (128, 4, D), chain, kind="ExternalInput")
    rhat_d = nc.dram_tensor("rhat", (128, 4, D), chain, kind="ExternalInput")
    # rows = [gext(=sx) | rhat_row512 | khat_row512 | m1row], all on part 0
    rows_d = nc.dram_tensor("rows", (1, 4 * D), chain, kind="ExternalInput")
    v4_d = nc.dram_tensor("v4", (128, 4), F32, kind="ExternalInput")
    out_d = nc.dram_tensor("out", (4, 128, NSLAB, D), BF16,
                           kind="ExternalOutput")

    with tile.TileContext(nc) as tc:
        with (
            tc.tile_pool(name="xa", bufs=4) as xa_pool,
            tc.tile_pool(name="const", bufs=1) as const_pool,
            tc.tile_pool(name="gsb", bufs=1) as g_pool,
            tc.tile_pool(name="chain", bufs=1) as chain_pool,
            tc.tile_pool(name="outsb", bufs=3) as out_pool,
        ):
            # ---- xa batch 0 on gpsimd: its preamble finishes first ----
            xa_tiles = []          # (tile, sub-index) per global row tile
            batch_dmas = []
            t0 = 0
            for bi, nb in enumerate(XA_BATCHES):
                xa_t = xa_pool.tile([128, 4, D], big, tag="xa")
                eng = nc.gpsimd if bi == 0 else nc.sync
                dma = eng.dma_start(xa_t[:, :nb, :], xa_d.ap()[:, t0:t0 + nb, :])
                batch_dmas.append(dma)
                for j in range(nb):
                    xa_tiles.append((xa_t, j))
                t0 += nb

            ident = const_pool.tile([128, 128], F32, tag="ident")
            make_identity(nc, ident[:])

            khat_sb = const_pool.tile([128, 4, D], chain, tag="khat")
            rhat_sb = const_pool.tile([128, 4, D], chain, tag="rhat")
            rows_sb = const_pool.tile([1, 4 * D], chain, tag="rows")
            v_sb = const_pool.tile([128, 4], F32, tag="v4")

            # ---- phase 1: G = x^T @ x; upper block-triangle only ----
            g_sb = [g_pool.tile([128, D], chain, tag=f"g{c}", name=f"g{c}")
                    for c in range(4)]
            with tc.tile_pool(name="psG", bufs=1, space="PSUM") as psG_pool:
                ps_ga = [psG_pool.tile([128, D - c * 128], F32, tag=f"ga{c}",
                                       name=f"ga{c}") for c in range(4)]
                gate_mms = []
                for t in range(NT):
                    xa_t, j = xa_tiles[t]
                    for c in range(4):
                        mm = nc.tensor.matmul(
                            ps_ga[c][:],
                            mm_ap(xa_t[:, j, c * 128:(c + 1) * 128], big),
                            mm_ap(xa_t[:, j, c * 128:D], big),
                            start=(t == 0), stop=(t == NT - 1),
                        )
                        if c == 3:
                            gate_mms.append(mm)

                # consts on gpsimd, gated behind early G so the xa stream
                # keeps priority during warmup
                const_dmas = [
                    nc.gpsimd.dma_start(khat_sb[:], khat_d.ap()[:]),
                    nc.gpsimd.dma_start(rhat_sb[:], rhat_d.ap()[:]),
                    nc.gpsimd.dma_start(rows_sb[:], rows_d.ap()[:]),
                    nc.gpsimd.dma_start(v_sb[:], v4_d.ap()[:]),
                ]
                for cd in const_dmas:
                    add_dep_helper(cd.ins, gate_mms[8].ins,
                                   reason="const loads gated behind G t=8")

                # xat loads (sync queue, after the xa triggers in program
                # order) gated behind the G tail: during G the xa stream +
                # consts saturate HBM; the chain window is otherwise idle.
                xat_sb = [const_pool.tile([128, NSLAB, D], big, tag=f"xat{c}",
                                          name=f"xat{c}") for c in range(4)]
                for h in range(2):
                    for c in range(4):
                        xd = nc.sync.dma_start(
                            xat_sb[c][:, 4 * h:4 * h + 4, :],
                            xat_d.ap()[c][:, 4 * h:4 * h + 4, :])
                        add_dep_helper(xd.ins, gate_mms[29 if h == 0 else 31].ins,
                                       reason="xat gated behind G tail")

                # upper blocks into SBUF
                for c in range(4):
                    nc.vector.tensor_copy(g_sb[c][:, c * 128:D], ps_ga[c][:])
                # lower blocks = transpose of upper (G symmetric)
                for c2 in range(1, 4):
                    for c1 in range(c2):
                        ps_tr = psG_pool.tile([128, 128], F32, tag="tr", bufs=2)
                        nc.tensor.transpose(
                            ps_tr[:],
                            mm_ap(g_sb[c1][:, c2 * 128:(c2 + 1) * 128], F32),
                            ident[:],
                        )
                        nc.vector.tensor_copy(
                            g_sb[c2][:, c1 * 128:(c1 + 1) * 128], ps_tr[:])

            # ---- phase 2: P = Khat @ G_aug @ Rhat (rank-1 aug terms enter
            # as K=1 matmuls from the host `rows`; P row 512 (=v) is fully
            # host-computed) ----
            gext = rows_sb[0:1, 0:D]
            rhat4 = rows_sb[0:1, D:2 * D]
            khat4 = rows_sb[0:1, 2 * D:3 * D]
            m1row = rows_sb[0:1, 3 * D:4 * D]
            with tc.tile_pool(name="psC", bufs=2, space="PSUM") as psC_pool:
                m1_sb = [chain_pool.tile([128, D], chain, tag=f"m1{c}",
                                         name=f"m1{c}") for c in range(4)]
                for g1 in range(4):
                    ps = psC_pool.tile([128, D], F32, tag="chain", bufs=3)
                    for g2 in range(4):
                        nc.tensor.matmul(
                            ps[:],
                            mm_ap(g_sb[g2][:, g1 * 128:(g1 + 1) * 128], chain),
                            mm_ap(rhat_sb[:, g2, :], chain),
                            start=(g2 == 0), stop=False,
                        )
                    nc.tensor.matmul(
                        ps[:], gext[:, g1 * 128:(g1 + 1) * 128], rhat4,
                        start=False, stop=True,
                    )
                    nc.vector.tensor_copy(m1_sb[g1][:], ps[:])

                p_sb = [chain_pool.tile([128, D], big, tag=f"p{c}",
                                        name=f"p{c}") for c in range(4)]
                for g1 in range(4):
                    ps = psC_pool.tile([128, D], F32, tag="chain", bufs=3)
                    for g2 in range(4):
                        nc.tensor.matmul(
                            ps[:],
                            mm_ap(khat_sb[:, g2, g1 * 128:(g1 + 1) * 128], chain),
                            mm_ap(m1_sb[g2][:], chain),
                            start=(g2 == 0), stop=False,
                        )
                    nc.tensor.matmul(
                        ps[:], khat4[:, g1 * 128:(g1 + 1) * 128], m1row,
                        start=False, stop=True,
                    )
                    nc.vector.tensor_copy(p_sb[g1][:], ps[:])

            # ---- phase 3: out.T[db,:] = sum_c P[c,db].T @ x.T[c,:] + v[db]
            # (+v fused into the Scalar-engine PSUM->SBUF copy as a
            # per-partition bias) ----
            with tc.tile_pool(name="psO", bufs=1, space="PSUM") as psO_pool:
                for h in range(2):
                    for db in range(4):
                        ot = out_pool.tile([128, 4, D], BF16, tag="ot")
                        for si in range(4):
                            s = 4 * h + si
                            ps = psO_pool.tile([128, D], F32, tag="out", bufs=6)
                            for c in range(4):
                                nc.tensor.matmul(
                                    ps[:],
                                    mm_ap(p_sb[c][:, db * 128:(db + 1) * 128], big),
                                    mm_ap(xat_sb[c][:, s, :], big),
                                    start=(c == 0), stop=(c == 3),
                                )
                            nc.scalar.add(ot[:, si, :], ps[:],
                                          v_sb[:, db:db + 1])
                        nc.gpsimd.dma_start(
                            out_d.ap()[db][:, 4 * h:4 * h + 4, :], ot[:])

    nc.compile()
    _built[mode] = nc
    return nc


def _prep_host(x, Wq1_w, Wq1_b, Wq2_w, Wq2_b, WR_w, WR_b, mode):
    f, f8 = np.float32, np.float64
    W1a = np.concatenate([Wq1_w, Wq1_b[:, None]], axis=1)   # [512, 513]
    W2a = np.concatenate([Wq2_w, Wq2_b[:, None]], axis=1)
    WRa = np.concatenate([WR_w, WR_b[:, None]], axis=1)

    khatT = (W2a.T.astype(f8) @ W1a.astype(f8)).astype(f)   # [513, 513]
    rhat = WRa.T.astype(f)                                  # [513, 512]
    khat2 = np.ascontiguousarray(
        khatT[:D, :D].reshape(4, 128, D).transpose(1, 0, 2))
    rhat2 = np.ascontiguousarray(
        rhat[:D].reshape(4, 128, D).transpose(1, 0, 2))

    sx = x.sum(axis=1, dtype=f8).astype(f)                  # [B, 512]
    sxa = np.concatenate([sx, np.full((B, 1), float(N), f)], axis=1)
    m1row = (sxa.astype(f8) @ rhat.astype(f8)).astype(f)    # [B, 512]
    rows = np.concatenate([
        sx,                                                  # gext
        np.broadcast_to(rhat[D], (B, D)),                    # Rhat row 512
        np.broadcast_to(khatT[D, :D], (B, D)),               # Khat[:,512].T
        m1row,
    ], axis=1).reshape(B, 1, 4 * D).astype(f)

    # v = P_aug[512,:] = Khat_aug[512,:] @ G_aug @ Rhat, host-computable in
    # O(N*D): z = x@k[:512] + k[512];  v = [x.T z | sum z] @ Rhat
    k = (W1a[:, D].astype(f8) @ W2a.astype(f8))             # [513]
    z = x.astype(f8) @ k[:D] + k[D]                         # [B, 4096]
    u = np.concatenate([np.einsum('bn,bnd->bd', z, x.astype(f8)),
                        z.sum(axis=1)[:, None]], axis=1)    # [B, 513]
    v = (u @ rhat.astype(f8)).astype(f)                     # [B, 512]
    v4 = np.ascontiguousarray(v.reshape(B, 4, 128).transpose(0, 2, 1))

    dt = ml_dtypes.bfloat16 if mode == "bf16" else f
    xa2 = np.ascontiguousarray(
        x.reshape(B, NT, 128, D).transpose(0, 2, 1, 3)).astype(dt)
    xat2 = np.ascontiguousarray(
        x.transpose(0, 2, 1).reshape(B, 4, 128, NSLAB, D)).astype(dt)

    return [
        {"xa": xa2[b], "xat": xat2[b], "khat": khat2, "rhat": rhat2,
         "rows": rows[b], "v4": v4[b]}
        for b in range(B)
    ]


def _post(res):
    # out.T tiles [db, p, slab, j] -> out[slab*512+j, db*128+p]
    return np.stack([
        np.ascontiguousarray(
            res.results[b]["out"].astype(np.float32)
            .transpose(2, 3, 0, 1)).reshape(N, D)
        for b in range(B)
    ])


def kernel(x, Wq1_w, Wq1_b, Wq2_w, Wq2_b, WR_w, WR_b):
    x = np.asarray(x, dtype=np.float32)
    args = [np.asarray(a, dtype=np.float32)
            for a in (Wq1_w, Wq1_b, Wq2_w, Wq2_b, WR_w, WR_b)]
    in_maps = _prep_host(x, *args, MODE)

    nc = _build(MODE)
    # the axon-tunneled device occasionally starts in a wedged state
    # (NRT_EXEC_UNIT_UNRECOVERABLE) and recovers on the next attempt
    last_err = None
    for attempt in range(3):
        try:
            res = run_bass_kernel_spmd(nc, in_maps, core_ids=list(range(N_CORES)))
            break
        except Exception as e:  # noqa: BLE001
            last_err = e
            import time as _time
            _time.sleep(2.0)
            try:
                import jax
                jax.clear_caches()
            except Exception:
                pass
    else:
        raise last_err
    return _post(res)


# revision 8
# speedup vs baseline: 1.3279x; 1.1776x over previous
"""Trainium2 Bass kernel for GCFAgg-style block:
    q1 = x@W1.T+b1; q2 = x@W2.T+b2; r = x@WR.T+br
    out = (q1 @ q2.T) @ r        (per batch, no softmax)

Key algebraic restructuring: with x_aug = [x | 1] and W*_aug = [W* | b*],
    out = x_aug @ (Khat @ (x_aug.T @ x_aug) @ Rhat)
where Khat = W1_aug.T @ W2_aug and Rhat = WR_aug.T are tiny host-precomputed
matrices. The device computes G = x.T @ x (symmetric: upper block-triangle
on PE, lower from PE transposes), the small chain P = Khat @ G @ Rhat, and
the projection out.T = P[:512].T @ x.T + v.

Work placement:
  - v (= P_aug row 512) is host-computed in O(N*D).
  - The rank-1 augmented terms of the chain (sx (x) rhat_row512 and
    khat_col512 (x) m1row) are materialized by the Scalar engine during the
    G window (scale-by-per-partition-scalar of a host-broadcast row) and
    folded into the chain's PSUM->SBUF copies as DVE adds — no K=1 PE
    matmuls.
  - G-symmetry: M1 groups run in order g1=3..0; group 3 needs only upper
    blocks, and each PE transpose that fills a lower block is interleaved
    right before the first group that consumes it.
  - out.T orientation makes +v a per-partition bias fused into the Scalar
    engine PSUM->SBUF copy; host reassembles the transposed output.

Perf notes (per core, PE @2.4GHz, ~332GB/s HBM):
  - PE ~127k cycles = 53us is the floor (G 41k, chain ~18k, out 65.5k).
  - bf16 x streams and P buy DMA bytes (bf16/f32r matmul are both
    1 cycle/row); the chain stays f32r for accuracy.
  - DMA triggers cost ~600ns of queue-engine time each regardless of size,
    so tiles move in large batched triggers; xa prefetch depth (pool bufs)
    covers all 10 triggers so the stream never backpressures.
  - A few warmup matmuls run during the initial DMA wait to pre-ramp the
    PE clock (0.65/1.2 GHz p-states before 3us of continuous work).

Sharding: batch dim B=8, one batch per NeuronCore (data parallel).

Self-contained: hardcodes shapes from the problem spec
(x: [8, 4096, 512] f32; W*: [512, 512]; b*: [512]).
"""
import os
import sys

sys.path.insert(0, "/opt/trn_rl_repo")

import numpy as np
import ml_dtypes

import concourse.bass as bass
import concourse.mybir as mybir
import concourse.tile as tile
from concourse import bacc
from concourse.bass_utils import run_bass_kernel_spmd
from concourse.masks import make_identity
from concourse.tile_rust import add_dep_helper

B = 8          # batch -> one per core
N = 4096       # tokens per batch
D = 512        # model dim
NT = N // 128  # 32 row tiles
NSLAB = 8      # 512-token slabs for the out.T phase
N_CORES = 8

F32 = mybir.dt.float32
F32R = mybir.dt.float32r
BF16 = mybir.dt.bfloat16

# mode: "bf16" (bf16 x/P storage+matmul, f32r chain) or "f32r"
MODE = os.environ.get("GCF_MODE", "bf16")

# xa trigger batching: first single tile goes on gpsimd (earliest-ready
# queue), the rest stream on sync
XA_BATCHES = [1, 1, 2, 4, 4, 4, 4, 4, 4, 4]
N_WARM = 4

_built = {}


def _build(mode):
    if mode in _built:
        return _built[mode]

    big = BF16 if mode == "bf16" else F32R
    chain = F32R

    def mm_ap(ap, dt):
        return ap if ap.dtype == dt else ap.bitcast(dt)

    nc = bacc.Bacc("TRN2", target_bir_lowering=False, debug=False,
                   num_devices=N_CORES)

    # all DRAM tensors are laid out partition-dim first by the host
    xa_d = nc.dram_tensor("xa", (128, NT, D), big, kind="ExternalInput")
    xat_d = nc.dram_tensor("xat", (4, 128, NSLAB, D), big, kind="ExternalInput")
    khat_d = nc.dram_tensor("khat", (128, 4, D), chain, kind="ExternalInput")
    rhat_d = nc.dram_tensor("rhat", (128, 4, D), chain, kind="ExternalInput")
    # scl[:, 0:4] = v (bias), [:, 4:8] = sx columns, [:, 8:12] = khat col 512
    scl_d = nc.dram_tensor("scl", (128, 12), F32, kind="ExternalInput")
    # broadcast rows: [0] = Rhat row 512, [1] = m1row (both repl. 128x)
    brow_d = nc.dram_tensor("brow", (128, 2, D), F32, kind="ExternalInput")
    out_d = nc.dram_tensor("out", (4, 128, NSLAB, D), BF16,
                           kind="ExternalOutput")

    with tile.TileContext(nc) as tc:
        with (
            tc.tile_pool(name="xa", bufs=len(XA_BATCHES)) as xa_pool,
            tc.tile_pool(name="const", bufs=1) as const_pool,
            tc.tile_pool(name="gsb", bufs=1) as g_pool,
            tc.tile_pool(name="chain", bufs=1) as chain_pool,
            tc.tile_pool(name="outsb", bufs=3) as out_pool,
        ):
            # ---- xa stream: batch 0 on gpsimd (its preamble ends first) ----
            xa_tiles = []          # (tile, sub-index) per global row tile
            t0 = 0
            for bi, nb in enumerate(XA_BATCHES):
                xa_t = xa_pool.tile([128, 4, D], big, tag="xa")
                eng = nc.gpsimd if bi == 0 else nc.sync
                eng.dma_start(xa_t[:, :nb, :], xa_d.ap()[:, t0:t0 + nb, :])
                for j in range(nb):
                    xa_tiles.append((xa_t, j))
                t0 += nb

            ident = const_pool.tile([128, 128], F32, tag="ident")
            make_identity(nc, ident[:])
            warm_sb = const_pool.tile([128, D], big, tag="warm")
            nc.vector.memset(warm_sb[:], 0.0)

            khat_sb = const_pool.tile([128, 4, D], chain, tag="khat")
            rhat_sb = const_pool.tile([128, 4, D], chain, tag="rhat")
            scl_sb = const_pool.tile([128, 12], F32, tag="scl")
            brow_sb = const_pool.tile([128, 2, D], F32, tag="brow")

            # ---- phase 1: G = x^T @ x; upper block-triangle only ----
            g_sb = [g_pool.tile([128, D], chain, tag=f"g{c}", name=f"g{c}")
                    for c in range(4)]
            with tc.tile_pool(name="psG", bufs=1, space="PSUM") as psG_pool:
                # warmup matmuls: pre-ramp the PE clock while the first xa
                # tiles are still in flight (results unused)
                ps_w = psG_pool.tile([128, D], F32, tag="warm")
                for _ in range(N_WARM):
                    nc.tensor.matmul(ps_w[:], mm_ap(warm_sb[:, :128], big),
                                     mm_ap(warm_sb[:], big),
                                     start=True, stop=True)

                ps_ga = [psG_pool.tile([128, D - c * 128], F32, tag=f"ga{c}",
                                       name=f"ga{c}") for c in range(4)]
                gate_mms = []
                for t in range(NT):
                    xa_t, j = xa_tiles[t]
                    for c in range(4):
                        mm = nc.tensor.matmul(
                            ps_ga[c][:],
                            mm_ap(xa_t[:, j, c * 128:(c + 1) * 128], big),
                            mm_ap(xa_t[:, j, c * 128:D], big),
                            start=(t == 0), stop=(t == NT - 1),
                        )
                        if c == 3:
                            gate_mms.append(mm)

                # consts on gpsimd, gated behind early G so the xa stream
                # keeps priority during warmup
                const_dmas = [
                    nc.gpsimd.dma_start(khat_sb[:], khat_d.ap()[:]),
                    nc.gpsimd.dma_start(rhat_sb[:], rhat_d.ap()[:]),
                    nc.gpsimd.dma_start(scl_sb[:], scl_d.ap()[:]),
                    nc.gpsimd.dma_start(brow_sb[:], brow_d.ap()[:]),
                ]
                for cd in const_dmas:
                    add_dep_helper(cd.ins, gate_mms[8].ins,
                                   reason="const loads gated behind G t=8")

                # xat loads (sync queue, after the xa triggers in program
                # order) gated behind the G tail: during G the xa stream +
                # consts saturate HBM; the chain window is otherwise idle.
                xat_sb = [const_pool.tile([128, NSLAB, D], big, tag=f"xat{c}",
                                          name=f"xat{c}") for c in range(4)]
                for h in range(2):
                    for c in range(4):
                        xd = nc.sync.dma_start(
                            xat_sb[c][:, 4 * h:4 * h + 4, :],
                            xat_d.ap()[c][:, 4 * h:4 * h + 4, :])
                        add_dep_helper(xd.ins, gate_mms[29 if h == 0 else 31].ins,
                                       reason="xat gated behind G tail")

                # ---- phase 2 interleaved with G wrap-up ----
                # upper-block PSUM->SBUF copies, most-urgent column first
                # (M1 group g1 consumes column block g1 of every row block)
                for g1 in range(3, -1, -1):
                    for c in range(g1 + 1):
                        nc.vector.tensor_copy(
                            g_sb[c][:, g1 * 128:(g1 + 1) * 128],
                            ps_ga[c][:, (g1 - c) * 128:(g1 - c + 1) * 128])

                gext_done = False
                with tc.tile_pool(name="psC", bufs=2, space="PSUM") as psC_pool:
                    m1_sb = [chain_pool.tile([128, D], chain, tag=f"m1{c}",
                                             name=f"m1{c}") for c in range(4)]

                    def transpose_block(c1, c2):
                        # fill lower block (c2, c1) from upper (c1, c2)
                        ps_tr = psC_pool.tile([128, 128], F32, tag="tr", bufs=1)
                        nc.tensor.transpose(
                            ps_tr[:],
                            mm_ap(g_sb[c1][:, c2 * 128:(c2 + 1) * 128], F32),
                            ident[:],
                        )
                        nc.vector.tensor_copy(
                            g_sb[c2][:, c1 * 128:(c1 + 1) * 128], ps_tr[:])

                    # M1 groups g1 = 3..0; PE transposes that fill the lower
                    # blocks needed by group g1 are emitted just before it
                    for g1 in range(3, -1, -1):
                        if g1 == 2:
                            transpose_block(2, 3)
                        elif g1 == 1:
                            transpose_block(1, 2)
                            transpose_block(1, 3)
                        elif g1 == 0:
                            transpose_block(0, 1)
                            transpose_block(0, 2)
                            transpose_block(0, 3)
                        ps = psC_pool.tile([128, D], F32, tag="chain", bufs=2)
                        for i, g2 in enumerate(
                                list(range(g1 + 1)) + list(range(g1 + 1, 4))):
                            nc.tensor.matmul(
                                ps[:],
                                mm_ap(g_sb[g2][:, g1 * 128:(g1 + 1) * 128],
                                      chain),
                                mm_ap(rhat_sb[:, g2, :], chain),
                                start=(i == 0), stop=(i == 3),
                            )
                        # m1 = ps + sx[g1-block] (x) Rhat[512,:] — the rank-1
                        # augmented term folds into the PSUM->SBUF copy
                        nc.vector.scalar_tensor_tensor(
                            m1_sb[g1][:], brow_sb[:, 0, :],
                            scl_sb[:, 4 + g1:5 + g1], ps[:],
                            mybir.AluOpType.mult, mybir.AluOpType.add)

                    p_sb = [chain_pool.tile([128, D], big, tag=f"p{c}",
                                            name=f"p{c}") for c in range(4)]
                    for g1 in range(4):
                        ps = psC_pool.tile([128, D], F32, tag="chain", bufs=2)
                        for g2 in range(4):
                            nc.tensor.matmul(
                                ps[:],
                                mm_ap(khat_sb[:, g2, g1 * 128:(g1 + 1) * 128],
                                      chain),
                                mm_ap(m1_sb[g2][:], chain),
                                start=(g2 == 0), stop=(g2 == 3),
                            )
                        # p = ps + Khat[g1-block, 512] (x) m1row
                        nc.vector.scalar_tensor_tensor(
                            p_sb[g1][:], brow_sb[:, 1, :],
                            scl_sb[:, 8 + g1:9 + g1], ps[:],
                            mybir.AluOpType.mult, mybir.AluOpType.add)

            # ---- phase 3: out.T[db,:] = sum_c P[c,db].T @ x.T[c,:] + v[db]
            # (+v fused into the Scalar-engine PSUM->SBUF copy as a
            # per-partition bias) ----
            with tc.tile_pool(name="psO", bufs=1, space="PSUM") as psO_pool:
                for h in range(2):
                    for db in range(4):
                        ot = out_pool.tile([128, 4, D], BF16, tag="ot")
                        for si in range(4):
                            s = 4 * h + si
                            ps = psO_pool.tile([128, D], F32, tag="out", bufs=6)
                            for c in range(4):
                                nc.tensor.matmul(
                                    ps[:],
                                    mm_ap(p_sb[c][:, db * 128:(db + 1) * 128],
                                          big),
                                    mm_ap(xat_sb[c][:, s, :], big),
                                    start=(c == 0), stop=(c == 3),
                                )
                            nc.scalar.add(ot[:, si, :], ps[:],
                                          scl_sb[:, db:db + 1])
                        nc.gpsimd.dma_start(
                            out_d.ap()[db][:, 4 * h:4 * h + 4, :], ot[:])

    nc.compile()
    _built[mode] = nc
    return nc


def _prep_host(x, Wq1_w, Wq1_b, Wq2_w, Wq2_b, WR_w, WR_b, mode):
    f, f8 = np.float32, np.float64
    W1a = np.concatenate([Wq1_w, Wq1_b[:, None]], axis=1)   # [512, 513]
    W2a = np.concatenate([Wq2_w, Wq2_b[:, None]], axis=1)
    WRa = np.concatenate([WR_w, WR_b[:, None]], axis=1)

    khatT = (W2a.T.astype(f8) @ W1a.astype(f8)).astype(f)   # [513, 513]
    rhat = WRa.T.astype(f)                                  # [513, 512]
    khat2 = np.ascontiguousarray(
        khatT[:D, :D].reshape(4, 128, D).transpose(1, 0, 2))
    rhat2 = np.ascontiguousarray(
        rhat[:D].reshape(4, 128, D).transpose(1, 0, 2))

    sx = x.sum(axis=1, dtype=f8).astype(f)                  # [B, 512]
    sxa = np.concatenate([sx, np.full((B, 1), float(N), f)], axis=1)
    m1row = (sxa.astype(f8) @ rhat.astype(f8)).astype(f)    # [B, 512]

    # v = P_aug[512,:] = Khat_aug[512,:] @ G_aug @ Rhat, host-computable in
    # O(N*D): z = x@k[:512] + k[512];  v = [x.T z | sum z] @ Rhat
    k = (W1a[:, D].astype(f8) @ W2a.astype(f8))             # [513]
    z = x.astype(f8) @ k[:D] + k[D]                         # [B, 4096]
    u = np.concatenate([np.einsum('bn,bnd->bd', z, x.astype(f8)),
                        z.sum(axis=1)[:, None]], axis=1)    # [B, 513]
    v = (u @ rhat.astype(f8)).astype(f)                     # [B, 512]

    # scl[:, 0:4] = v, [:, 4:8] = sx, [:, 8:12] = Khat[:, 512], col-major
    scl = np.concatenate([
        v.reshape(B, 4, 128), sx.reshape(B, 4, 128),
        np.broadcast_to(khatT[D, :D].reshape(1, 4, 128), (B, 4, 128)),
    ], axis=1).transpose(0, 2, 1).astype(f)                 # [B, 128, 12]
    scl = np.ascontiguousarray(scl)
    brow = np.stack([
        np.broadcast_to(rhat[D], (B, 128, D)),
        np.repeat(m1row[:, None, :], 128, axis=1),
    ], axis=2).astype(f)                                    # [B, 128, 2, D]
    brow = np.ascontiguousarray(brow)

    dt = ml_dtypes.bfloat16 if mode == "bf16" else f
    xa2 = np.ascontiguousarray(
        x.reshape(B, NT, 128, D).transpose(0, 2, 1, 3)).astype(dt)
    xat2 = np.ascontiguousarray(
        x.transpose(0, 2, 1).reshape(B, 4, 128, NSLAB, D)).astype(dt)

    return [
        {"xa": xa2[b], "xat": xat2[b], "khat": khat2, "rhat": rhat2,
         "scl": scl[b], "brow": brow[b]}
        for b in range(B)
    ]


def _post(res):
    # out.T tiles [db, p, slab, j] -> out[slab*512+j, db*128+p]
    return np.stack([
        np.ascontiguousarray(
            res.results[b]["out"].astype(np.float32)
            .transpose(2, 3, 0, 1)).reshape(N, D)
        for b in range(B)
    ])


def kernel(x, Wq1_w, Wq1_b, Wq2_w, Wq2_b, WR_w, WR_b):
    x = np.asarray(x, dtype=np.float32)
    args = [np.asarray(a, dtype=np.float32)
            for a in (Wq1_w, Wq1_b, Wq2_w, Wq2_b, WR_w, WR_b)]
    in_maps = _prep_host(x, *args, MODE)

    nc = _build(MODE)
    # the axon-tunneled device occasionally starts in a wedged state
    # (NRT_EXEC_UNIT_UNRECOVERABLE) and recovers on the next attempt
    last_err = None
    for attempt in range(3):
        try:
            res = run_bass_kernel_spmd(nc, in_maps, core_ids=list(range(N_CORES)))
            break
        except Exception as e:  # noqa: BLE001
            last_err = e
            import time as _time
            _time.sleep(2.0)
            try:
                import jax
                jax.clear_caches()
            except Exception:
                pass
    else:
        raise last_err
    return _post(res)


# revision 12
# speedup vs baseline: 1.3662x; 1.0289x over previous
"""Trainium2 Bass kernel for GCFAgg-style block:
    q1 = x@W1.T+b1; q2 = x@W2.T+b2; r = x@WR.T+br
    out = (q1 @ q2.T) @ r        (per batch, no softmax)

Key algebraic restructuring: with x_aug = [x | 1] and W*_aug = [W* | b*],
    out = x_aug @ (Khat @ (x_aug.T @ x_aug) @ Rhat)
where Khat = W1_aug.T @ W2_aug and Rhat = WR_aug.T are tiny host-precomputed
matrices. The device computes G = x.T @ x (symmetric: upper block-triangle
on PE, lower from PE transposes), the small chain P = Khat @ G @ Rhat, and
the projection out.T = P[:512].T @ x.T + v.

Work placement:
  - v (= P_aug row 512) is host-computed in O(N*D).
  - The rank-1 augmented terms of the chain (sx (x) rhat_row512 and
    khat_col512 (x) m1row) are materialized by the Scalar engine during the
    G window (scale-by-per-partition-scalar of a host-broadcast row) and
    folded into the chain's PSUM->SBUF copies as DVE adds — no K=1 PE
    matmuls.
  - G-symmetry: M1 groups run in order g1=3..0; group 3 needs only upper
    blocks, and each PE transpose that fills a lower block is interleaved
    right before the first group that consumes it.
  - out.T orientation makes +v a per-partition bias fused into the Scalar
    engine PSUM->SBUF copy; host reassembles the transposed output.

Perf notes (per core, PE @2.4GHz, ~332GB/s HBM):
  - PE ~127k cycles = 53us is the floor (G 41k, chain ~18k, out 65.5k).
  - bf16 x streams and P buy DMA bytes (bf16/f32r matmul are both
    1 cycle/row); the chain stays f32r for accuracy.
  - DMA triggers cost ~600ns of queue-engine time each regardless of size,
    so tiles move in large batched triggers; xa prefetch depth (pool bufs)
    covers all 10 triggers so the stream never backpressures.
  - A few warmup matmuls run during the initial DMA wait to pre-ramp the
    PE clock (0.65/1.2 GHz p-states before 3us of continuous work).

Sharding: batch dim B=8, one batch per NeuronCore (data parallel).

Self-contained: hardcodes shapes from the problem spec
(x: [8, 4096, 512] f32; W*: [512, 512]; b*: [512]).
"""
import os
import sys

sys.path.insert(0, "/opt/trn_rl_repo")

import numpy as np
import ml_dtypes

import concourse.bass as bass
import concourse.mybir as mybir
import concourse.tile as tile
from concourse import bacc
from concourse.bass_utils import run_bass_kernel_spmd
from concourse.masks import make_identity
from concourse.tile_rust import add_dep_helper

B = 8          # batch -> one per core
N = 4096       # tokens per batch
D = 512        # model dim
NT = N // 128  # 32 row tiles
NSLAB = 8      # 512-token slabs for the out.T phase
N_CORES = 8

F32 = mybir.dt.float32
F32R = mybir.dt.float32r
BF16 = mybir.dt.bfloat16

# mode: "bf16" (bf16 x/P storage+matmul, f32r chain) or "f32r"
MODE = os.environ.get("GCF_MODE", "bf16")

# xa trigger batching: first single tile goes on gpsimd (earliest-ready
# queue), the rest stream on sync
XA_BATCHES = [1, 1, 2, 4, 4, 4, 4, 4, 4, 4]
N_WARM = 10

_built = {}


def _build(mode):
    if mode in _built:
        return _built[mode]

    big = BF16 if mode == "bf16" else F32R
    chain = F32R

    def mm_ap(ap, dt):
        return ap if ap.dtype == dt else ap.bitcast(dt)

    nc = bacc.Bacc("TRN2", target_bir_lowering=False, debug=False,
                   num_devices=N_CORES)

    # all DRAM tensors are laid out partition-dim first by the host
    xa_d = nc.dram_tensor("xa", (128, NT, D), big, kind="ExternalInput")
    xat_d = nc.dram_tensor("xat", (4, 128, NSLAB, D), big, kind="ExternalInput")
    khat_d = nc.dram_tensor("khat", (128, 4, D), chain, kind="ExternalInput")
    rhat_d = nc.dram_tensor("rhat", (128, 4, D), chain, kind="ExternalInput")
    # scl[:, 0:4] = v (bias), [:, 4:8] = sx columns, [:, 8:12] = khat col 512
    scl_d = nc.dram_tensor("scl", (128, 12), F32, kind="ExternalInput")
    # broadcast rows: [0] = Rhat row 512, [1] = m1row (both repl. 128x)
    brow_d = nc.dram_tensor("brow", (128, 2, D), F32, kind="ExternalInput")
    out_d = nc.dram_tensor("out", (4, 128, NSLAB, D), BF16,
                           kind="ExternalOutput")

    with tile.TileContext(nc) as tc:
        with (
            tc.tile_pool(name="xa", bufs=len(XA_BATCHES)) as xa_pool,
            tc.tile_pool(name="const", bufs=1) as const_pool,
            tc.tile_pool(name="gsb", bufs=1) as g_pool,
            tc.tile_pool(name="chain", bufs=1) as chain_pool,
            tc.tile_pool(name="outsb", bufs=3) as out_pool,
        ):
            # ---- xa stream: batch 0 on gpsimd (its preamble ends first) ----
            xa_tiles = []          # (tile, sub-index) per global row tile
            t0 = 0
            for bi, nb in enumerate(XA_BATCHES):
                xa_t = xa_pool.tile([128, 4, D], big, tag="xa")
                eng = nc.gpsimd if bi == 0 else nc.sync
                eng.dma_start(xa_t[:, :nb, :], xa_d.ap()[:, t0:t0 + nb, :])
                for j in range(nb):
                    xa_tiles.append((xa_t, j))
                t0 += nb

            ident = const_pool.tile([128, 128], F32, tag="ident")
            make_identity(nc, ident[:])
            warm_sb = const_pool.tile([128, D], big, tag="warm")
            nc.vector.memset(warm_sb[:], 0.0)

            khat_sb = const_pool.tile([128, 4, D], chain, tag="khat")
            rhat_sb = const_pool.tile([128, 4, D], chain, tag="rhat")
            scl_sb = const_pool.tile([128, 12], F32, tag="scl")
            brow_sb = const_pool.tile([128, 2, D], F32, tag="brow")

            # ---- phase 1: G = x^T @ x; upper block-triangle only ----
            g_sb = [g_pool.tile([128, D], chain, tag=f"g{c}", name=f"g{c}")
                    for c in range(4)]
            with tc.tile_pool(name="psG", bufs=1, space="PSUM") as psG_pool:
                # warmup matmuls: pre-ramp the PE clock while the first xa
                # tiles are still in flight (results unused)
                ps_w = psG_pool.tile([128, D], F32, tag="warm")
                for _ in range(N_WARM):
                    nc.tensor.matmul(ps_w[:], mm_ap(warm_sb[:, :128], big),
                                     mm_ap(warm_sb[:], big),
                                     start=True, stop=True)

                ps_ga = [psG_pool.tile([128, D - c * 128], F32, tag=f"ga{c}",
                                       name=f"ga{c}") for c in range(4)]
                gate_mms = []
                for t in range(NT):
                    xa_t, j = xa_tiles[t]
                    for c in range(4):
                        mm = nc.tensor.matmul(
                            ps_ga[c][:],
                            mm_ap(xa_t[:, j, c * 128:(c + 1) * 128], big),
                            mm_ap(xa_t[:, j, c * 128:D], big),
                            start=(t == 0), stop=(t == NT - 1),
                        )
                        if c == 3:
                            gate_mms.append(mm)

                # consts on gpsimd, gated behind early G so the xa stream
                # keeps priority during warmup
                const_dmas = [
                    nc.gpsimd.dma_start(khat_sb[:], khat_d.ap()[:]),
                    nc.gpsimd.dma_start(rhat_sb[:], rhat_d.ap()[:]),
                    nc.gpsimd.dma_start(scl_sb[:], scl_d.ap()[:]),
                    nc.gpsimd.dma_start(brow_sb[:], brow_d.ap()[:]),
                ]
                for cd in const_dmas:
                    add_dep_helper(cd.ins, gate_mms[8].ins,
                                   reason="const loads gated behind G t=8")

                # xat loads (sync queue, after the xa triggers in program
                # order) gated behind the G tail: during G the xa stream +
                # consts saturate HBM; the chain window is otherwise idle.
                xat_sb = [const_pool.tile([128, NSLAB, D], big, tag=f"xat{c}",
                                          name=f"xat{c}") for c in range(4)]
                for h in range(2):
                    for c in range(4):
                        xd = nc.sync.dma_start(
                            xat_sb[c][:, 4 * h:4 * h + 4, :],
                            xat_d.ap()[c][:, 4 * h:4 * h + 4, :])
                        add_dep_helper(xd.ins, gate_mms[29 if h == 0 else 31].ins,
                                       reason="xat gated behind G tail")

                # ---- phase 2 interleaved with G wrap-up; the upper-block
                # PSUM->SBUF copies are emitted just-in-time per column so
                # the DVE backlog never stalls the next M1 group ----
                def copy_col(g1):
                    for c in range(g1 + 1):
                        nc.vector.tensor_copy(
                            g_sb[c][:, g1 * 128:(g1 + 1) * 128],
                            ps_ga[c][:, (g1 - c) * 128:(g1 - c + 1) * 128])

                with tc.tile_pool(name="psC", bufs=2, space="PSUM") as psC_pool:
                    m1_sb = [chain_pool.tile([128, D], chain, tag=f"m1{c}",
                                             name=f"m1{c}") for c in range(4)]

                    def transpose_block(c1, c2):
                        # fill lower block (c2, c1) from upper (c1, c2)
                        ps_tr = psC_pool.tile([128, 128], F32, tag="tr", bufs=1)
                        nc.tensor.transpose(
                            ps_tr[:],
                            mm_ap(g_sb[c1][:, c2 * 128:(c2 + 1) * 128], F32),
                            ident[:],
                        )
                        nc.vector.tensor_copy(
                            g_sb[c2][:, c1 * 128:(c1 + 1) * 128], ps_tr[:])

                    # M1 groups g1 = 3..0; PE transposes that fill the lower
                    # blocks needed by group g1 are emitted just before it
                    for g1 in range(3, -1, -1):
                        copy_col(g1)
                        if g1 == 2:
                            transpose_block(2, 3)
                        elif g1 == 1:
                            transpose_block(1, 2)
                            transpose_block(1, 3)
                        elif g1 == 0:
                            transpose_block(0, 1)
                            transpose_block(0, 2)
                            transpose_block(0, 3)
                        ps = psC_pool.tile([128, D], F32, tag="chain", bufs=2)
                        for i, g2 in enumerate(
                                list(range(g1 + 1)) + list(range(g1 + 1, 4))):
                            nc.tensor.matmul(
                                ps[:],
                                mm_ap(g_sb[g2][:, g1 * 128:(g1 + 1) * 128],
                                      chain),
                                mm_ap(rhat_sb[:, g2, :], chain),
                                start=(i == 0), stop=(i == 3),
                            )
                        # m1 = ps + sx[g1-block] (x) Rhat[512,:] — the rank-1
                        # augmented term folds into the PSUM->SBUF copy
                        nc.vector.scalar_tensor_tensor(
                            m1_sb[g1][:], brow_sb[:, 0, :],
                            scl_sb[:, 4 + g1:5 + g1], ps[:],
                            mybir.AluOpType.mult, mybir.AluOpType.add)

                    p_sb = [chain_pool.tile([128, D], big, tag=f"p{c}",
                                            name=f"p{c}") for c in range(4)]
                    for g1 in range(4):
                        ps = psC_pool.tile([128, D], F32, tag="chain", bufs=2)
                        for g2 in range(4):
                            nc.tensor.matmul(
                                ps[:],
                                mm_ap(khat_sb[:, g2, g1 * 128:(g1 + 1) * 128],
                                      chain),
                                mm_ap(m1_sb[g2][:], chain),
                                start=(g2 == 0), stop=(g2 == 3),
                            )
                        # p = ps + Khat[g1-block, 512] (x) m1row
                        nc.vector.scalar_tensor_tensor(
                            p_sb[g1][:], brow_sb[:, 1, :],
                            scl_sb[:, 8 + g1:9 + g1], ps[:],
                            mybir.AluOpType.mult, mybir.AluOpType.add)

            # ---- phase 3: out.T[db,:] = sum_c P[c,db].T @ x.T[c,:] + v[db]
            # (+v fused into the Scalar-engine PSUM->SBUF copy as a
            # per-partition bias) ----
            with tc.tile_pool(name="psO", bufs=1, space="PSUM") as psO_pool:
                nst = 0
                for h in range(2):
                    for db in range(4):
                        ot = out_pool.tile([128, 4, D], BF16, tag="ot")
                        for si in range(4):
                            s = 4 * h + si
                            ps = psO_pool.tile([128, D], F32, tag="out", bufs=6)
                            for c in range(4):
                                nc.tensor.matmul(
                                    ps[:],
                                    mm_ap(p_sb[c][:, db * 128:(db + 1) * 128],
                                          big),
                                    mm_ap(xat_sb[c][:, s, :], big),
                                    start=(c == 0), stop=(c == 3),
                                )
                            nc.scalar.add(ot[:, si, :], ps[:],
                                          scl_sb[:, db:db + 1])
                            # split the final buffer's store in two so the
                            # last transfer off the critical path is short
                            if h == 1 and db == 3 and si in (1, 3):
                                eng = nc.gpsimd if nst % 2 == 0 else nc.sync
                                nst += 1
                                lo = 4 * h + (0 if si == 1 else 2)
                                eng.dma_start(
                                    out_d.ap()[db][:, lo:lo + 2, :],
                                    ot[:, (0 if si == 1 else 2):
                                       (2 if si == 1 else 4), :])
                        if not (h == 1 and db == 3):
                            eng = nc.gpsimd if nst % 2 == 0 else nc.sync
                            nst += 1
                            eng.dma_start(
                                out_d.ap()[db][:, 4 * h:4 * h + 4, :], ot[:])

    nc.compile()
    _built[mode] = nc
    return nc


def _prep_host(x, Wq1_w, Wq1_b, Wq2_w, Wq2_b, WR_w, WR_b, mode):
    f, f8 = np.float32, np.float64
    W1a = np.concatenate([Wq1_w, Wq1_b[:, None]], axis=1)   # [512, 513]
    W2a = np.concatenate([Wq2_w, Wq2_b[:, None]], axis=1)
    WRa = np.concatenate([WR_w, WR_b[:, None]], axis=1)

    khatT = (W2a.T.astype(f8) @ W1a.astype(f8)).astype(f)   # [513, 513]
    rhat = WRa.T.astype(f)                                  # [513, 512]
    khat2 = np.ascontiguousarray(
        khatT[:D, :D].reshape(4, 128, D).transpose(1, 0, 2))
    rhat2 = np.ascontiguousarray(
        rhat[:D].reshape(4, 128, D).transpose(1, 0, 2))

    sx = x.sum(axis=1, dtype=f8).astype(f)                  # [B, 512]
    sxa = np.concatenate([sx, np.full((B, 1), float(N), f)], axis=1)
    m1row = (sxa.astype(f8) @ rhat.astype(f8)).astype(f)    # [B, 512]

    # v = P_aug[512,:] = Khat_aug[512,:] @ G_aug @ Rhat, host-computable in
    # O(N*D): z = x@k[:512] + k[512];  v = [x.T z | sum z] @ Rhat
    k = (W1a[:, D].astype(f8) @ W2a.astype(f8))             # [513]
    z = x.astype(f8) @ k[:D] + k[D]                         # [B, 4096]
    u = np.concatenate([np.einsum('bn,bnd->bd', z, x.astype(f8)),
                        z.sum(axis=1)[:, None]], axis=1)    # [B, 513]
    v = (u @ rhat.astype(f8)).astype(f)                     # [B, 512]

    # scl[:, 0:4] = v, [:, 4:8] = sx, [:, 8:12] = Khat[:, 512], col-major
    scl = np.concatenate([
        v.reshape(B, 4, 128), sx.reshape(B, 4, 128),
        np.broadcast_to(khatT[D, :D].reshape(1, 4, 128), (B, 4, 128)),
    ], axis=1).transpose(0, 2, 1).astype(f)                 # [B, 128, 12]
    scl = np.ascontiguousarray(scl)
    brow = np.stack([
        np.broadcast_to(rhat[D], (B, 128, D)),
        np.repeat(m1row[:, None, :], 128, axis=1),
    ], axis=2).astype(f)                                    # [B, 128, 2, D]
    brow = np.ascontiguousarray(brow)

    dt = ml_dtypes.bfloat16 if mode == "bf16" else f
    xa2 = np.ascontiguousarray(
        x.reshape(B, NT, 128, D).transpose(0, 2, 1, 3)).astype(dt)
    xat2 = np.ascontiguousarray(
        x.transpose(0, 2, 1).reshape(B, 4, 128, NSLAB, D)).astype(dt)

    return [
        {"xa": xa2[b], "xat": xat2[b], "khat": khat2, "rhat": rhat2,
         "scl": scl[b], "brow": brow[b]}
        for b in range(B)
    ]


def _post(res):
    # out.T tiles [db, p, slab, j] -> out[slab*512+j, db*128+p]
    return np.stack([
        np.ascontiguousarray(
            res.results[b]["out"].astype(np.float32)
            .transpose(2, 3, 0, 1)).reshape(N, D)
        for b in range(B)
    ])


def kernel(x, Wq1_w, Wq1_b, Wq2_w, Wq2_b, WR_w, WR_b):
    x = np.asarray(x, dtype=np.float32)
    args = [np.asarray(a, dtype=np.float32)
            for a in (Wq1_w, Wq1_b, Wq2_w, Wq2_b, WR_w, WR_b)]
    in_maps = _prep_host(x, *args, MODE)

    nc = _build(MODE)
    # the axon-tunneled device occasionally starts in a wedged state
    # (NRT_EXEC_UNIT_UNRECOVERABLE) and recovers on the next attempt
    last_err = None
    for attempt in range(3):
        try:
            res = run_bass_kernel_spmd(nc, in_maps, core_ids=list(range(N_CORES)))
            break
        except Exception as e:  # noqa: BLE001
            last_err = e
            import time as _time
            _time.sleep(2.0)
            try:
                import jax
                jax.clear_caches()
            except Exception:
                pass
    else:
        raise last_err
    return _post(res)


# revision 18
# speedup vs baseline: 1.4219x; 1.0407x over previous
"""Trainium2 Bass kernel for GCFAgg-style block:
    q1 = x@W1.T+b1; q2 = x@W2.T+b2; r = x@WR.T+br
    out = (q1 @ q2.T) @ r        (per batch, no softmax)

Key algebraic restructuring: with x_aug = [x | 1] and W*_aug = [W* | b*],
    out = x_aug @ (Khat @ (x_aug.T @ x_aug) @ Rhat)
where Khat = W1_aug.T @ W2_aug and Rhat = WR_aug.T are tiny host-precomputed
matrices. The device computes G = x.T @ x (symmetric: upper block-triangle
on PE, lower from PE transposes), the small chain P = Khat @ G @ Rhat, and
the projection out.T = P[:512].T @ x.T + v.

Work placement:
  - v (= P_aug row 512) is host-computed in O(N*D).
  - The rank-1 augmented terms of the chain (sx (x) rhat_row512 and
    khat_col512 (x) m1row) are materialized by the Scalar engine during the
    G window (scale-by-per-partition-scalar of a host-broadcast row) and
    folded into the chain's PSUM->SBUF copies as DVE adds — no K=1 PE
    matmuls.
  - G-symmetry: M1 groups run in order g1=3..0; group 3 needs only upper
    blocks, and each PE transpose that fills a lower block is interleaved
    right before the first group that consumes it.
  - out.T orientation makes +v a per-partition bias fused into the Scalar
    engine PSUM->SBUF copy; host reassembles the transposed output.

Perf notes (per core, PE @2.4GHz, ~332GB/s HBM):
  - PE ~127k cycles = 53us is the floor (G 41k, chain ~18k, out 65.5k).
  - bf16 x streams and P buy DMA bytes (bf16/f32r matmul are both
    1 cycle/row); the chain stays f32r for accuracy.
  - DMA triggers cost ~600ns of queue-engine time each regardless of size,
    so tiles move in large batched triggers; xa prefetch depth (pool bufs)
    covers all 10 triggers so the stream never backpressures.
  - A few warmup matmuls run during the initial DMA wait to pre-ramp the
    PE clock (0.65/1.2 GHz p-states before 3us of continuous work).

Sharding: batch dim B=8, one batch per NeuronCore (data parallel).

Self-contained: hardcodes shapes from the problem spec
(x: [8, 4096, 512] f32; W*: [512, 512]; b*: [512]).
"""
import os
import sys

sys.path.insert(0, "/opt/trn_rl_repo")

import numpy as np
import ml_dtypes

import concourse.bass as bass
import concourse.mybir as mybir
import concourse.tile as tile
from concourse import bacc
from concourse.bass_utils import run_bass_kernel_spmd
from concourse.masks import make_identity
from concourse.tile_rust import add_dep_helper

B = 8          # batch -> one per core
N = 4096       # tokens per batch
D = 512        # model dim
NT = N // 128  # 32 row tiles
NSLAB = 8      # 512-token slabs for the out.T phase
N_CORES = 8

F32 = mybir.dt.float32
F32R = mybir.dt.float32r
BF16 = mybir.dt.bfloat16

# mode: "bf16" (bf16 x/P storage+matmul, f32r chain) or "f32r"
MODE = os.environ.get("GCF_MODE", "bf16")

# xa trigger batching: first single tile goes on gpsimd (earliest-ready
# queue), the rest stream on sync
XA_BATCHES = [1, 1, 2, 4, 4, 4, 4, 4, 4, 4]
N_WARM = 10

_built = {}


def _build(mode):
    if mode in _built:
        return _built[mode]

    big = BF16 if mode == "bf16" else F32R
    chain = F32R

    def mm_ap(ap, dt):
        return ap if ap.dtype == dt else ap.bitcast(dt)

    nc = bacc.Bacc("TRN2", target_bir_lowering=False, debug=False,
                   num_devices=N_CORES)

    # all DRAM tensors are laid out partition-dim first by the host
    xa_d = nc.dram_tensor("xa", (128, NT, D), big, kind="ExternalInput")
    xat_d = nc.dram_tensor("xat", (4, 128, NSLAB, D), big, kind="ExternalInput")
    khat_d = nc.dram_tensor("khat", (128, 4, D), chain, kind="ExternalInput")
    rhat_d = nc.dram_tensor("rhat", (128, 4, D), chain, kind="ExternalInput")
    # scl[:, 0:4] = v (bias), [:, 4:8] = sx columns, [:, 8:12] = khat col 512
    scl_d = nc.dram_tensor("scl", (128, 12), F32, kind="ExternalInput")
    # broadcast rows: [0] = Rhat row 512, [1] = m1row (both repl. 128x)
    brow_d = nc.dram_tensor("brow", (128, 2, D), F32, kind="ExternalInput")
    out_d = nc.dram_tensor("out", (4, 128, NSLAB, D), BF16,
                           kind="ExternalOutput")

    with tile.TileContext(nc) as tc:
        with (
            tc.tile_pool(name="xa", bufs=len(XA_BATCHES)) as xa_pool,
            tc.tile_pool(name="const", bufs=1) as const_pool,
            tc.tile_pool(name="gsb", bufs=1) as g_pool,
            tc.tile_pool(name="chain", bufs=1) as chain_pool,
            tc.tile_pool(name="outsb", bufs=3) as out_pool,
        ):
            # ---- xa stream: batch 0 on gpsimd (its preamble ends first) ----
            xa_tiles = []          # (tile, sub-index) per global row tile
            t0 = 0
            for bi, nb in enumerate(XA_BATCHES):
                xa_t = xa_pool.tile([128, 4, D], big, tag="xa")
                eng = nc.gpsimd if bi == 0 else nc.sync
                eng.dma_start(xa_t[:, :nb, :], xa_d.ap()[:, t0:t0 + nb, :])
                for j in range(nb):
                    xa_tiles.append((xa_t, j))
                t0 += nb

            ident = const_pool.tile([128, 128], F32, tag="ident")
            make_identity(nc, ident[:])
            warm_sb = const_pool.tile([128, D], big, tag="warm")
            nc.vector.memset(warm_sb[:], 0.0)

            khat_sb = const_pool.tile([128, 4, D], chain, tag="khat")
            rhat_sb = const_pool.tile([128, 4, D], chain, tag="rhat")
            scl_sb = const_pool.tile([128, 12], F32, tag="scl")
            brow_sb = const_pool.tile([128, 2, D], F32, tag="brow")

            # ---- phase 1: G = x^T @ x; upper block-triangle only ----
            g_sb = [g_pool.tile([128, D], chain, tag=f"g{c}", name=f"g{c}")
                    for c in range(4)]
            with tc.tile_pool(name="psG", bufs=1, space="PSUM") as psG_pool:
                # warmup matmuls: pre-ramp the PE clock while the first xa
                # tiles are still in flight (results unused)
                ps_w = psG_pool.tile([128, D], F32, tag="warm")
                for _ in range(N_WARM):
                    nc.tensor.matmul(ps_w[:], mm_ap(warm_sb[:, :128], big),
                                     mm_ap(warm_sb[:], big),
                                     start=True, stop=True)

                ps_ga = [psG_pool.tile([128, D - c * 128], F32, tag=f"ga{c}",
                                       name=f"ga{c}") for c in range(4)]
                gate_mms = []
                for t in range(NT):
                    xa_t, j = xa_tiles[t]
                    for c in range(4):
                        mm = nc.tensor.matmul(
                            ps_ga[c][:],
                            mm_ap(xa_t[:, j, c * 128:(c + 1) * 128], big),
                            mm_ap(xa_t[:, j, c * 128:D], big),
                            start=(t == 0), stop=(t == NT - 1),
                        )
                        if c == 3:
                            gate_mms.append(mm)

                # consts on gpsimd, gated behind early G so the xa stream
                # keeps priority during warmup
                const_dmas = [
                    nc.gpsimd.dma_start(khat_sb[:], khat_d.ap()[:]),
                    nc.gpsimd.dma_start(rhat_sb[:], rhat_d.ap()[:]),
                    nc.gpsimd.dma_start(scl_sb[:], scl_d.ap()[:]),
                    nc.gpsimd.dma_start(brow_sb[:], brow_d.ap()[:]),
                ]
                for cd in const_dmas:
                    add_dep_helper(cd.ins, gate_mms[8].ins,
                                   reason="const loads gated behind G t=8")

                # xat loads (sync queue, after the xa triggers in program
                # order) gated behind the G tail: during G the xa stream +
                # consts saturate HBM; the chain window is otherwise idle.
                xat_sb = [const_pool.tile([128, NSLAB, D], big, tag=f"xat{c}",
                                          name=f"xat{c}") for c in range(4)]
                for h in range(2):
                    for c in range(4):
                        xd = nc.sync.dma_start(
                            xat_sb[c][:, 4 * h:4 * h + 4, :],
                            xat_d.ap()[c][:, 4 * h:4 * h + 4, :])
                        add_dep_helper(xd.ins, gate_mms[29 if h == 0 else 31].ins,
                                       reason="xat gated behind G tail")

                # ---- phase 2 interleaved with G wrap-up; the upper-block
                # PSUM->SBUF copies are emitted just-in-time per column so
                # the DVE backlog never stalls the next M1 group ----
                # the copies run on the Scalar engine (idle until phase 3,
                # and it CAN read PSUM) so the DVE queue holds only the
                # chain adds — neither engine's backlog stalls the PE
                def copy_col(g1):
                    for c in range(g1 + 1):
                        nc.scalar.copy(
                            g_sb[c][:, g1 * 128:(g1 + 1) * 128],
                            ps_ga[c][:, (g1 - c) * 128:(g1 - c + 1) * 128])

                with tc.tile_pool(name="psC", bufs=2, space="PSUM") as psC_pool:
                    m1_sb = [chain_pool.tile([128, D], chain, tag=f"m1{c}",
                                             name=f"m1{c}") for c in range(4)]

                    def transpose_block(c1, c2):
                        # fill lower block (c2, c1) from upper (c1, c2)
                        ps_tr = psC_pool.tile([128, 128], F32, tag="tr", bufs=1)
                        nc.tensor.transpose(
                            ps_tr[:],
                            mm_ap(g_sb[c1][:, c2 * 128:(c2 + 1) * 128], F32),
                            ident[:],
                        )
                        nc.scalar.copy(
                            g_sb[c2][:, c1 * 128:(c1 + 1) * 128], ps_tr[:])

                    # M1 groups g1 = 3..0; PE transposes that fill the lower
                    # blocks needed by group g1 are emitted just before it
                    for g1 in range(3, -1, -1):
                        copy_col(g1)
                        if g1 == 2:
                            transpose_block(2, 3)
                        elif g1 == 1:
                            transpose_block(1, 2)
                            transpose_block(1, 3)
                        elif g1 == 0:
                            transpose_block(0, 1)
                            transpose_block(0, 2)
                            transpose_block(0, 3)
                        ps = psC_pool.tile([128, D], F32, tag="chain", bufs=2)
                        for i, g2 in enumerate(
                                list(range(g1 + 1)) + list(range(g1 + 1, 4))):
                            nc.tensor.matmul(
                                ps[:],
                                mm_ap(g_sb[g2][:, g1 * 128:(g1 + 1) * 128],
                                      chain),
                                mm_ap(rhat_sb[:, g2, :], chain),
                                start=(i == 0), stop=(i == 3),
                            )
                        # m1 = ps + sx[g1-block] (x) Rhat[512,:] — the rank-1
                        # augmented term folds into the PSUM->SBUF copy
                        nc.vector.scalar_tensor_tensor(
                            m1_sb[g1][:], brow_sb[:, 0, :],
                            scl_sb[:, 4 + g1:5 + g1], ps[:],
                            mybir.AluOpType.mult, mybir.AluOpType.add)

                    p_sb = [chain_pool.tile([128, D], big, tag=f"p{c}",
                                            name=f"p{c}") for c in range(4)]
                    for g1 in range(4):
                        ps = psC_pool.tile([128, D], F32, tag="chain", bufs=2)
                        for g2 in range(4):
                            nc.tensor.matmul(
                                ps[:],
                                mm_ap(khat_sb[:, g2, g1 * 128:(g1 + 1) * 128],
                                      chain),
                                mm_ap(m1_sb[g2][:], chain),
                                start=(g2 == 0), stop=(g2 == 3),
                            )
                        # p = ps + Khat[g1-block, 512] (x) m1row
                        nc.vector.scalar_tensor_tensor(
                            p_sb[g1][:], brow_sb[:, 1, :],
                            scl_sb[:, 8 + g1:9 + g1], ps[:],
                            mybir.AluOpType.mult, mybir.AluOpType.add)

            # ---- phase 3: out.T[db,:] = sum_c P[c,db].T @ x.T[c,:] + v[db]
            # (+v fused into the Scalar-engine PSUM->SBUF copy as a
            # per-partition bias) ----
            with tc.tile_pool(name="psO", bufs=1, space="PSUM") as psO_pool:
                nst = 0
                for h in range(2):
                    for db in range(4):
                        ot = out_pool.tile([128, 4, D], BF16, tag="ot")
                        for si in range(4):
                            s = 4 * h + si
                            ps = psO_pool.tile([128, D], F32, tag="out", bufs=6)
                            for c in range(4):
                                nc.tensor.matmul(
                                    ps[:],
                                    mm_ap(p_sb[c][:, db * 128:(db + 1) * 128],
                                          big),
                                    mm_ap(xat_sb[c][:, s, :], big),
                                    start=(c == 0), stop=(c == 3),
                                )
                            nc.scalar.add(ot[:, si, :], ps[:],
                                          scl_sb[:, db:db + 1])
                            # split the final buffer's store in two so the
                            # last transfer off the critical path is short
                            if h == 1 and db == 3 and si in (1, 3):
                                eng = nc.gpsimd if nst % 2 == 0 else nc.sync
                                nst += 1
                                lo = 4 * h + (0 if si == 1 else 2)
                                eng.dma_start(
                                    out_d.ap()[db][:, lo:lo + 2, :],
                                    ot[:, (0 if si == 1 else 2):
                                       (2 if si == 1 else 4), :])
                        if not (h == 1 and db == 3):
                            eng = nc.gpsimd if nst % 2 == 0 else nc.sync
                            nst += 1
                            eng.dma_start(
                                out_d.ap()[db][:, 4 * h:4 * h + 4, :], ot[:])

    nc.compile()
    _built[mode] = nc
    return nc


def _prep_host(x, Wq1_w, Wq1_b, Wq2_w, Wq2_b, WR_w, WR_b, mode):
    f, f8 = np.float32, np.float64
    W1a = np.concatenate([Wq1_w, Wq1_b[:, None]], axis=1)   # [512, 513]
    W2a = np.concatenate([Wq2_w, Wq2_b[:, None]], axis=1)
    WRa = np.concatenate([WR_w, WR_b[:, None]], axis=1)

    khatT = (W2a.T.astype(f8) @ W1a.astype(f8)).astype(f)   # [513, 513]
    rhat = WRa.T.astype(f)                                  # [513, 512]
    khat2 = np.ascontiguousarray(
        khatT[:D, :D].reshape(4, 128, D).transpose(1, 0, 2))
    rhat2 = np.ascontiguousarray(
        rhat[:D].reshape(4, 128, D).transpose(1, 0, 2))

    sx = x.sum(axis=1, dtype=f8).astype(f)                  # [B, 512]
    sxa = np.concatenate([sx, np.full((B, 1), float(N), f)], axis=1)
    m1row = (sxa.astype(f8) @ rhat.astype(f8)).astype(f)    # [B, 512]

    # v = P_aug[512,:] = Khat_aug[512,:] @ G_aug @ Rhat, host-computable in
    # O(N*D): z = x@k[:512] + k[512];  v = [x.T z | sum z] @ Rhat
    k = (W1a[:, D].astype(f8) @ W2a.astype(f8))             # [513]
    z = x.astype(f8) @ k[:D] + k[D]                         # [B, 4096]
    u = np.concatenate([np.einsum('bn,bnd->bd', z, x.astype(f8)),
                        z.sum(axis=1)[:, None]], axis=1)    # [B, 513]
    v = (u @ rhat.astype(f8)).astype(f)                     # [B, 512]

    # scl[:, 0:4] = v, [:, 4:8] = sx, [:, 8:12] = Khat[:, 512], col-major
    scl = np.concatenate([
        v.reshape(B, 4, 128), sx.reshape(B, 4, 128),
        np.broadcast_to(khatT[D, :D].reshape(1, 4, 128), (B, 4, 128)),
    ], axis=1).transpose(0, 2, 1).astype(f)                 # [B, 128, 12]
    scl = np.ascontiguousarray(scl)
    brow = np.stack([
        np.broadcast_to(rhat[D], (B, 128, D)),
        np.repeat(m1row[:, None, :], 128, axis=1),
    ], axis=2).astype(f)                                    # [B, 128, 2, D]
    brow = np.ascontiguousarray(brow)

    dt = ml_dtypes.bfloat16 if mode == "bf16" else f
    xa2 = np.ascontiguousarray(
        x.reshape(B, NT, 128, D).transpose(0, 2, 1, 3)).astype(dt)
    xat2 = np.ascontiguousarray(
        x.transpose(0, 2, 1).reshape(B, 4, 128, NSLAB, D)).astype(dt)

    return [
        {"xa": xa2[b], "xat": xat2[b], "khat": khat2, "rhat": rhat2,
         "scl": scl[b], "brow": brow[b]}
        for b in range(B)
    ]


def _post(res):
    # out.T tiles [db, p, slab, j] -> out[slab*512+j, db*128+p]
    return np.stack([
        np.ascontiguousarray(
            res.results[b]["out"].astype(np.float32)
            .transpose(2, 3, 0, 1)).reshape(N, D)
        for b in range(B)
    ])


def kernel(x, Wq1_w, Wq1_b, Wq2_w, Wq2_b, WR_w, WR_b):
    x = np.asarray(x, dtype=np.float32)
    args = [np.asarray(a, dtype=np.float32)
            for a in (Wq1_w, Wq1_b, Wq2_w, Wq2_b, WR_w, WR_b)]
    in_maps = _prep_host(x, *args, MODE)

    nc = _build(MODE)
    # the axon-tunneled device occasionally starts in a wedged state
    # (NRT_EXEC_UNIT_UNRECOVERABLE) and recovers on the next attempt
    last_err = None
    for attempt in range(3):
        try:
            res = run_bass_kernel_spmd(nc, in_maps, core_ids=list(range(N_CORES)))
            break
        except Exception as e:  # noqa: BLE001
            last_err = e
            import time as _time
            _time.sleep(2.0)
            try:
                import jax
                jax.clear_caches()
            except Exception:
                pass
    else:
        raise last_err
    return _post(res)
